# revision 1
# baseline (speedup 1.0000x reference)
"""Trainium2 Bass kernel for nn_ActorCritic loss_fn.

Strategy
--------
The reference computes a reverse discounted-return scan over time (T=8192),
normalizes the returns by masked global mean/std, and reduces to two scalar
losses. Both losses are polynomial in 10 masked global sums involving the raw
(unnormalized) returns R:

    N     = sum(m)          S1   = sum(m*R)       S2   = sum(m*R^2)
    SV    = sum(m*V)        SRV  = sum(m*R*V)     SV2  = sum(m*V^2)
    SLP   = sum(m*lp)       SLPR = sum(m*lp*R)    SLPV = sum(m*lp*V)
    SE    = sum(m*e)

so the device kernel is ONE streaming pass: compute R on the fly, form masked
products, reduce. Final scalar math happens on host in float64.

Sharding: batch dim split 8 ways -> (8192, 512) per core, streamed once.

Per-core pipeline (time tiled into 64 chunks of 128 rows = partition dim):
  SP  : streams inputs per 512-row super-chunk, double-buffered.
        rewards as f32(r); V/lp/e/mask pre-cast to bf16 on host.
  PE  : reverse scan per chunk as fp32r matmul with a lower-triangular
        gamma-power matrix; cross-chunk carry as a second fp32r matmul with a
        row-selector matrix (Sel[q,p] = gamma^(P-p) iff q==0) against the
        previous chunk's f32 R tile, accumulated into the same PSUM bank.
        Also: 8 of the 10 stat reductions as bf16 ones-column matmuls
        accumulating into a shared PSUM stats bank across all 64 chunks.
  ACT : copies R PSUM->SBUF twice (f32r for the carry chain, bf16 for the
        products) and does 2 stat reductions via activation accum_out.
  DVE : 7 bf16 elementwise products (2x perf mode).
  GPS : 2 bf16 elementwise products.

Raw Bass with manual semaphores and standalone wait_ge instructions: this
container's walrus build allows only ONE sync wait per instruction, which
rules out the Tile layer's fused-wait style.

Numerics: products in bf16 with f32 accumulation; scan in fp32r (~2e-5 rms).
Expected end-to-end relative error ~1e-5 vs the f32 reference.
"""

import numpy as np
from contextlib import ExitStack

GAMMA = 0.99
ALPHA = 0.01
EPS = 1e-8

T = 8192
B = 4096
NCORES = 8
BL = B // NCORES        # 512 batch columns per core
P = 128                 # time rows per scan chunk (SBUF partition dim)
KPC = 4                 # chunks per DMA super-chunk (512 rows)
NSUPER = T // (P * KPC)  # 16
NCHUNK = T // P          # 64

# PE-reduced stats (rows of the PSUM stats bank, via ones-column matmuls)
PE_STATS = ("N", "S1", "SV", "SLP", "S2", "SRV", "SLPR", "SV2")
NPE = len(PE_STATS)
# ACT-reduced stats (activation accum_out, per-chunk columns in `acc`)
ACT_STATS = ("SE", "SLPV")
NACT = len(ACT_STATS)

_cache = {}


def _build_program():
    import concourse.bass as bass
    import concourse.mybir as mybir

    dt = mybir.dt
    f32 = dt.float32
    f32r = dt.float32r
    bf16 = dt.bfloat16
    mult = mybir.AluOpType.mult
    Copy = mybir.ActivationFunctionType.Copy

    nc = bass.Bass()
    r_d = nc.dram_tensor("rewards", [T, BL], f32r, kind="ExternalInput")
    v_d = nc.dram_tensor("value_estimates", [T, BL], bf16, kind="ExternalInput")
    l_d = nc.dram_tensor("log_probs", [T, BL], bf16, kind="ExternalInput")
    e_d = nc.dram_tensor("entropies", [T, BL], bf16, kind="ExternalInput")
    m_d = nc.dram_tensor("to_include", [T, BL], bf16, kind="ExternalInput")
    acc_d = nc.dram_tensor("acc_out", [P, NACT * NCHUNK], f32, kind="ExternalOutput")
    pes_d = nc.dram_tensor("pe_stats", [NPE, BL], f32, kind="ExternalOutput")

    qi = np.arange(P)
    # scan lhsT[q, p] = gamma^(q-p) for q >= p (lower triangular)
    scan_np = np.tril(GAMMA ** (qi[:, None] - qi[None, :])).astype(np.float32)
    scan_d = nc.inline_tensor(scan_np, "scanmat")
    # carry selector lhsT[q, p] = gamma^(P-p) iff q == 0:
    # out[p, n] = gamma^(P-p) * R_next[0, n]
    sel_np = np.zeros((P, P), dtype=np.float32)
    sel_np[0, :] = GAMMA ** (P - qi)
    sel_d = nc.inline_tensor(sel_np, "selmat")
    # ones-column matrices for the PE stat reductions: oneh[:, j*NPE + k] = (k == j)
    import ml_dtypes
    oneh_np = np.zeros((P, NPE * NPE), dtype=np.float32)
    for j in range(NPE):
        oneh_np[:, j * NPE + j] = 1.0
    oneh_d = nc.inline_tensor(oneh_np.astype(ml_dtypes.bfloat16), "onehmat")

    with ExitStack() as ctx:
        def sb(name, shape, dtype):
            return ctx.enter_context(nc.sbuf_tensor(name, list(shape), dtype))

        scan_sb = sb("scan_sb", (P, P), f32r)
        sel_sb = sb("sel_sb", (P, P), f32r)
        oneh_sb = sb("oneh_sb", (P, NPE * NPE), bf16)
        r4 = [sb(f"r4_{i}", (P, KPC, BL), f32r) for i in range(2)]
        v4 = [sb(f"v4_{i}", (P, KPC, BL), bf16) for i in range(2)]
        l4 = [sb(f"l4_{i}", (P, KPC, BL), bf16) for i in range(2)]
        e4 = [sb(f"e4_{i}", (P, KPC, BL), bf16) for i in range(2)]
        m4 = [sb(f"m4_{i}", (P, KPC, BL), bf16) for i in range(2)]
        R_sb = [sb(f"R_sb_{i}", (P, BL), f32r) for i in range(3)]
        R_bf = [sb(f"R_bf_{i}", (P, KPC, BL), bf16) for i in range(2)]
        # DVE products, batched per super-chunk (double-buffered by super parity)
        mR = [sb(f"mR_{i}", (P, KPC, BL), bf16) for i in range(2)]
        mV = [sb(f"mV_{i}", (P, KPC, BL), bf16) for i in range(2)]
        mL = [sb(f"mL_{i}", (P, KPC, BL), bf16) for i in range(2)]
        pRR = [sb(f"pRR_{i}", (P, KPC, BL), bf16) for i in range(2)]
        pRV = [sb(f"pRV_{i}", (P, KPC, BL), bf16) for i in range(2)]
        pLR = [sb(f"pLR_{i}", (P, KPC, BL), bf16) for i in range(2)]
        pLV = [sb(f"pLV_{i}", (P, KPC, BL), bf16) for i in range(2)]
        # GPS products
        pME = [sb(f"pME_{i}", (P, KPC, BL), bf16) for i in range(2)]
        pVV = [sb(f"pVV_{i}", (P, KPC, BL), bf16) for i in range(2)]
        acc = sb("acc", (P, NACT * NCHUNK), f32)
        stats_sb = sb("stats_sb", (NPE, BL), f32)
        R_ps = [ctx.enter_context(nc.psum_tensor(f"R_ps_{i}", [P, BL], f32))
                for i in range(2)]
        st_ps = ctx.enter_context(nc.psum_tensor("st_ps", [NPE, BL], f32))

        def acol(stat, c):
            col = ACT_STATS.index(stat) * NCHUNK + c
            return acc[:, col:col + 1]

        def nsame(s):
            return (NSUPER - 1 - s) // 2 + 1

        with nc.Block() as block, \
                nc.semaphore("const_sem") as const_sem, \
                nc.semaphore("dma_even") as dma_even, \
                nc.semaphore("dma_odd") as dma_odd, \
                nc.semaphore("pe_scan") as pe_scan, \
                nc.semaphore("pe_done") as pe_done, \
                nc.semaphore("act_rc") as act_rc, \
                nc.semaphore("act_red") as act_red, \
                nc.semaphore("dve_l1") as dve_l1, \
                nc.semaphore("dve_l2") as dve_l2, \
                nc.semaphore("gps_done") as gps_done, \
                nc.semaphore("act_fin") as act_fin, \
                nc.semaphore("dma_out") as dma_out:
            dma_par = (dma_even, dma_odd)

            @block.sync
            def _(sync):
                sync.dma_start(out=scan_sb[:], in_=scan_d[:].bitcast(f32r)).then_inc(const_sem, 16)
                sync.dma_start(out=sel_sb[:], in_=sel_d[:].bitcast(f32r)).then_inc(const_sem, 16)
                sync.dma_start(out=oneh_sb[:], in_=oneh_d[:]).then_inc(const_sem, 16)
                for s in reversed(range(NSUPER)):
                    if s <= NSUPER - 3:
                        done = NCHUNK - KPC * (s + 2)
                        sync.wait_ge(pe_done, done)      # PE reduce groups (r4, m4)
                        sync.wait_ge(dve_l1, NSUPER - 2 - s)   # DVE level-1 (v4, l4, m4)
                        sync.wait_ge(gps_done, 2 * (NSUPER - 2 - s))  # GPS (m4, e4)
                    sl = s % 2
                    rows = slice(s * P * KPC, (s + 1) * P * KPC)
                    for dst, src in ((r4[sl], r_d), (v4[sl], v_d), (l4[sl], l_d),
                                     (e4[sl], e_d), (m4[sl], m_d)):
                        sync.dma_start(
                            out=dst[:],
                            in_=src[rows, :].rearrange("(k p) n -> p k n", p=P),
                        ).then_inc(dma_par[sl], 16)
                sync.wait_ge(act_red, NACT * NCHUNK)
                sync.wait_ge(act_fin, 1)
                sync.dma_start(out=acc_d[:], in_=acc[:]).then_inc(dma_out, 16)
                sync.dma_start(out=pes_d[:], in_=stats_sb[:]).then_inc(dma_out, 16)
                sync.wait_ge(dma_out, 32)

            def pe_reduces(pe, c):
                """stat-reduction matmuls for chunk c (emitted 4 iters later)"""
                s, k = divmod(c, KPC)
                sl = s % 2
                pe.wait_ge(dve_l2, NSUPER - s)
                pe.wait_ge(gps_done, 2 * (NSUPER - s))
                srcs = {
                    "N": m4[sl][:, k, :], "S1": mR[sl][:, k, :], "SV": mV[sl][:, k, :],
                    "SLP": mL[sl][:, k, :], "S2": pRR[sl][:, k, :], "SRV": pRV[sl][:, k, :],
                    "SLPR": pLR[sl][:, k, :], "SV2": pVV[sl][:, k, :],
                }
                start = c == NCHUNK - 1
                for j, stat in enumerate(PE_STATS):
                    mm = pe.matmul(st_ps[:], lhsT=oneh_sb[:, j * NPE:(j + 1) * NPE],
                                   rhs=srcs[stat],
                                   start=(start and j == 0),
                                   stop=(c == 0 and j == NPE - 1))
                    if stat == PE_STATS[-1]:
                        mm.then_inc(pe_done, 1)

            @block.tensor
            def _(pe):
                pe.wait_ge(const_sem, 48)
                for c in reversed(range(NCHUNK)):
                    s, k = divmod(c, KPC)
                    if k == KPC - 1:
                        pe.wait_ge(dma_par[s % 2], 80 * nsame(s))
                    if c <= NCHUNK - 3:
                        # R_ps bank c%2 must be fully drained by ACT (conv of c+2)
                        pe.wait_ge(act_rc, 2 * (NCHUNK - 2 - c))
                    rv = r4[s % 2][:, k, :]
                    ps = R_ps[c % 2]
                    if c == NCHUNK - 1:
                        mm = pe.matmul(ps[:], lhsT=scan_sb[:], rhs=rv,
                                       start=True, stop=True)
                    else:
                        pe.matmul(ps[:], lhsT=scan_sb[:], rhs=rv,
                                  start=True, stop=False)
                        # R_sb[c+1] written by ACT copy (odd act_rc increments)
                        pe.wait_ge(act_rc, 2 * (NCHUNK - 2 - c) + 1)
                        mm = pe.matmul(ps[:], lhsT=sel_sb[:],
                                       rhs=R_sb[(c + 1) % 3][:],
                                       start=False, stop=True)
                    mm.then_inc(pe_scan, 1)
                    if c < NCHUNK - 4:
                        pe_reduces(pe, c + 4)
                for c in (3, 2, 1, 0):
                    pe_reduces(pe, c)

            def act_reduces(act, c):
                # in-place copies: out == in avoids an unsynced scratch tile;
                # the accum_out is the real result
                s, k = divmod(c, KPC)
                sl = s % 2
                act.wait_ge(gps_done, 2 * (NSUPER - s) - 1)  # pME of super s
                act.activation(pME[sl][:, k, :], pME[sl][:, k, :], Copy,
                               accum_out=acol("SE", c)).then_inc(act_red, 1)
                act.wait_ge(dve_l2, NSUPER - s)
                act.activation(pLV[sl][:, k, :], pLV[sl][:, k, :], Copy,
                               accum_out=acol("SLPV", c)).then_inc(act_red, 1)

            @block.scalar
            def _(act):
                for c in reversed(range(NCHUNK)):
                    act.wait_ge(pe_scan, NCHUNK - c)
                    # R_sb slot WAR: rank1 of c+2 read slot (c+3)%3 == c%3
                    # covered by pe_scan wait above (rank1(c) done => rank1(c+2) done)
                    act.activation(R_sb[c % 3][:], R_ps[c % 2][:], Copy) \
                        .then_inc(act_rc, 1)
                    s, k = divmod(c, KPC)
                    if k == KPC - 1 and s <= NSUPER - 3:
                        # R_bf slot WAR: DVE level-1 of super s+2 read R_bf[s%2]
                        act.wait_ge(dve_l1, NSUPER - 2 - s)
                    act.activation(R_bf[s % 2][:, k, :], R_ps[c % 2][:], Copy) \
                        .then_inc(act_rc, 1)
                    if c < NCHUNK - 4:
                        act_reduces(act, c + 4)
                for cc in (3, 2, 1, 0):
                    act_reduces(act, cc)
                act.wait_ge(pe_done, NCHUNK)
                act.activation(stats_sb[:], st_ps[:], Copy).then_inc(act_fin, 1)

            @block.vector
            def _(dve):
                for s in reversed(range(NSUPER)):
                    sl = s % 2
                    dve.wait_ge(dma_par[sl], 80 * nsame(s))
                    # R_bf for all 4 chunks of super s (conv of chunk 4s last)
                    dve.wait_ge(act_rc, 2 * (NCHUNK - KPC * s))
                    if s <= NSUPER - 3:
                        # product tiles (sl) reused from super s+2: readers
                        dve.wait_ge(pe_done, NCHUNK - KPC * (s + 2))
                        dve.wait_ge(act_red, NACT * (NCHUNK - KPC * (s + 2)))
                        dve.wait_ge(gps_done, 2 * (NSUPER - 2 - s))
                    mv_in = m4[sl][:]
                    dve.tensor_tensor(out=mR[sl][:], in0=mv_in, in1=R_bf[sl][:], op=mult)
                    dve.tensor_tensor(out=mV[sl][:], in0=mv_in, in1=v4[sl][:], op=mult)
                    dve.tensor_tensor(out=mL[sl][:], in0=mv_in, in1=l4[sl][:], op=mult) \
                        .then_inc(dve_l1, 1)
                    # self-wait on dve_l1 orders level-2 after the level-1
                    # writes are committed (much cheaper than a full DRAIN)
                    dve.wait_ge(dve_l1, NSUPER - s)
                    dve.tensor_tensor(out=pRR[sl][:], in0=mR[sl][:], in1=mR[sl][:], op=mult)
                    dve.tensor_tensor(out=pRV[sl][:], in0=mR[sl][:], in1=mV[sl][:], op=mult)
                    dve.tensor_tensor(out=pLR[sl][:], in0=mL[sl][:], in1=mR[sl][:], op=mult)
                    dve.tensor_tensor(out=pLV[sl][:], in0=mL[sl][:], in1=mV[sl][:], op=mult) \
                        .then_inc(dve_l2, 1)

            @block.gpsimd
            def _(gps):
                for s in reversed(range(NSUPER)):
                    sl = s % 2
                    gps.wait_ge(dma_par[sl], 80 * nsame(s))
                    if s <= NSUPER - 3:
                        # pME/pVV tiles reused from super s+2: readers ACT, PE
                        gps.wait_ge(act_red, NACT * (NCHUNK - KPC * (s + 2)))
                        gps.wait_ge(pe_done, NCHUNK - KPC * (s + 2))
                    gps.tensor_tensor(out=pME[sl][:], in0=m4[sl][:],
                                      in1=e4[sl][:], op=mult).then_inc(gps_done, 1)
                    gps.wait_ge(dve_l1, NSUPER - s)
                    gps.tensor_tensor(out=pVV[sl][:], in0=mV[sl][:],
                                      in1=mV[sl][:], op=mult).then_inc(gps_done, 1)

    return nc


def _get_program():
    if "nc" not in _cache:
        _cache["nc"] = _build_program()
    return _cache["nc"]


def _shard_inputs(inputs):
    import ml_dtypes

    bf16 = ml_dtypes.bfloat16
    r = np.ascontiguousarray(inputs["rewards"], dtype=np.float32)
    v = np.asarray(inputs["value_estimates"], dtype=np.float32).astype(bf16)
    lp = np.asarray(inputs["log_probs"], dtype=np.float32).astype(bf16)
    e = np.asarray(inputs["entropies"], dtype=np.float32).astype(bf16)
    m = inputs["to_include"].astype(bf16)
    in_maps = []
    for c in range(NCORES):
        sl = slice(c * BL, (c + 1) * BL)
        in_maps.append({
            "rewards": np.ascontiguousarray(r[:, sl]),
            "value_estimates": np.ascontiguousarray(v[:, sl]),
            "log_probs": np.ascontiguousarray(lp[:, sl]),
            "entropies": np.ascontiguousarray(e[:, sl]),
            "to_include": np.ascontiguousarray(m[:, sl]),
        })
    return in_maps


def _execute(in_maps, trace=False):
    from concourse.bass_utils import run_bass_kernel_spmd

    nc = _get_program()
    res = run_bass_kernel_spmd(nc, in_maps, list(range(NCORES)), trace=trace)
    return res


def _stats_from_results(results):
    tot = {name: 0.0 for name in PE_STATS + ACT_STATS}
    for cm in results:
        pes = cm["pe_stats"].astype(np.float64)
        for j, name in enumerate(PE_STATS):
            tot[name] += pes[j].sum()
        ac = cm["acc_out"].astype(np.float64)
        for i, name in enumerate(ACT_STATS):
            tot[name] += ac[:, i * NCHUNK:(i + 1) * NCHUNK].sum()
    return tot


def _finalize(tot):
    N = tot["N"]; S1 = tot["S1"]; S2 = tot["S2"]
    SV = tot["SV"]; SRV = tot["SRV"]; SV2 = tot["SV2"]
    SLP = tot["SLP"]; SLPR = tot["SLPR"]; SLPV = tot["SLPV"]; SE = tot["SE"]
    mean = S1 / N
    q = S2 - 2.0 * mean * S1 + mean * mean * N   # sum(m*(R-mean)^2)
    var = q / (N - 1.0)
    s = np.sqrt(var) + EPS
    critic = q / (s * s) - 2.0 * (SRV - mean * SV) / s + SV2
    actor = -(SLPR - mean * SLP) / s + SLPV - ALPHA * SE
    return (np.float32(critic), np.float32(actor))


def kernel(**inputs):
    in_maps = _shard_inputs(inputs)
    res = _execute(in_maps, trace=False)
    tot = _stats_from_results(res.results)
    return _finalize(tot)



# revision 2
# speedup vs baseline: 1.4225x; 1.4225x over previous
"""Trainium2 Bass kernel for nn_ActorCritic loss_fn.

Strategy (v2, batch-major)
--------------------------
Both losses are polynomials in 10 masked global sums over the discounted
returns R, values V, log-probs L, entropies E and mask m:

    N    = sum(m)        S1   = sum(m*R)      S2  = sum(m*R^2)
    SV   = sum(m*V)      SRV  = sum(m*R*V)    SV2 = sum(m*V^2)
    SLP  = sum(m*L)      SLPR = sum(m*L*R)    SLPV= sum(m*L*V)
    SE   = sum(m*E)

Layout: batch on partitions, time along the free dim, TIME-REVERSED on the
host.  Each core gets 512 batch columns = 4 partition-blocks of 128; each
block's 8192 time steps split into 4 windows of 2048 -> 16 units of
(128, 2048) per core, streamed with a 3-deep ring.

Engines per unit:
  DVE : discounted-return scan as a native tensor_tensor_scan
        (state = gamma*state + r, fp32 state, gamma held as an f32 tile so
        the recurrence matches the reference bit-for-bit in structure),
        chained across windows via initial=prev[:, -1:]; then bf16 2x
        products mR, mV, mL, mRV, mLR (and mLV on some units).
  GPS : mE product every unit; mLV product on the other units.
  PE  : 7 stat reductions (N,S1,SV,SLP,SRV,SLPR,SLPV) as ones-column
        matmuls accumulating into one PSUM bank across all units.
  ACT : Square+accum_out for S2 and SV2, Copy+accum_out for SE
        (one column per unit; host sums).

Raw Bass with manual semaphores (walrus build allows one sync wait per
instruction -> standalone wait_ge).  Final scalar math on host in float64.
"""

import numpy as np
from contextlib import ExitStack

GAMMA = 0.99
ALPHA = 0.01
EPS = 1e-8

T = 8192
B = 4096
NCORES = 8
BL = B // NCORES          # 512 batch columns per core
P = 128                   # partition dim (batch block)
NBLK = BL // P            # 4 batch blocks
W = 2048                  # time window (free dim per unit)
NWIN = T // W             # 4 windows per block
NUNIT = NBLK * NWIN       # 16 units, u = j*NWIN + w
NCH = W // 512            # 4 matmul chunks per unit (moving max 512)

# dtypes for rewards / entropies ("bf16" or "fp8")
R_DT = "fp8"
E_DT = "fp8"
# units whose mLV product runs on GPS instead of DVE
LV_ON_GPS = [u % 2 == 1 for u in range(NUNIT)]

PE_STATS = ("N", "S1", "SV", "SLP", "SRV", "SLPR", "SLPV")
NPE = len(PE_STATS)

_cache = {}


def _build_program():
    import concourse.bass as bass
    import concourse.mybir as mybir
    import ml_dtypes

    dt = mybir.dt
    f32 = dt.float32
    bf16 = dt.bfloat16
    fp8 = dt.float8e4
    mult = mybir.AluOpType.mult
    add = mybir.AluOpType.add
    Square = mybir.ActivationFunctionType.Square
    Copy = mybir.ActivationFunctionType.Copy

    r_dt = fp8 if R_DT == "fp8" else bf16
    e_dt = fp8 if E_DT == "fp8" else bf16

    nc = bass.Bass()
    r_d = nc.dram_tensor("rewards", [NUNIT * P, W], r_dt, kind="ExternalInput")
    v_d = nc.dram_tensor("value_estimates", [NUNIT * P, W], bf16, kind="ExternalInput")
    l_d = nc.dram_tensor("log_probs", [NUNIT * P, W], bf16, kind="ExternalInput")
    e_d = nc.dram_tensor("entropies", [NUNIT * P, W], e_dt, kind="ExternalInput")
    m_d = nc.dram_tensor("to_include", [NUNIT * P, W], bf16, kind="ExternalInput")
    pes_d = nc.dram_tensor("pe_stats", [NPE, BL], f32, kind="ExternalOutput")
    cols_d = nc.dram_tensor("acc_cols", [P, 3 * NUNIT], f32, kind="ExternalOutput")

    # onehot matrix for stat matmuls: oneh[:, j*NPE + j] = 1
    oneh_np = np.zeros((P, NPE * NPE), dtype=np.float32)
    for j in range(NPE):
        oneh_np[:, j * NPE + j] = 1.0
    oneh_d = nc.inline_tensor(oneh_np.astype(ml_dtypes.bfloat16), "onehmat")
    # gamma tile for the scan (f32 so the recurrence coefficient is exact)
    gam_d = nc.inline_tensor(np.full((P, W), GAMMA, dtype=np.float32), "gammat")

    with ExitStack() as ctx:
        def sb(name, shape, dtype):
            return ctx.enter_context(nc.sbuf_tensor(name, list(shape), dtype))

        oneh_sb = sb("oneh_sb", (P, NPE * NPE), bf16)
        gam_sb = sb("gam_sb", (P, W), f32)
        r_in = [sb(f"r_in{i}", (P, W), r_dt) for i in range(3)]
        v_in = [sb(f"v_in{i}", (P, W), bf16) for i in range(3)]
        l_in = [sb(f"l_in{i}", (P, W), bf16) for i in range(3)]
        e_in = [sb(f"e_in{i}", (P, W), e_dt) for i in range(3)]
        m_in = [sb(f"m_in{i}", (P, W), bf16) for i in range(3)]
        R_t = [sb(f"R_t{i}", (P, W), bf16) for i in range(2)]
        mR = [sb(f"mR{i}", (P, W), bf16) for i in range(2)]
        mV = [sb(f"mV{i}", (P, W), bf16) for i in range(2)]
        mL = [sb(f"mL{i}", (P, W), bf16) for i in range(2)]
        mRV = [sb(f"mRV{i}", (P, W), bf16) for i in range(2)]
        mLR = [sb(f"mLR{i}", (P, W), bf16) for i in range(2)]
        mLV = [sb(f"mLV{i}", (P, W), bf16) for i in range(2)]
        mE = [sb(f"mE{i}", (P, W), bf16) for i in range(2)]
        sq = sb("sq", (P, W), bf16)
        cols = sb("cols", (P, 3 * NUNIT), f32)
        stats_sb = sb("stats_sb", (NPE, BL), f32)
        st_ps = ctx.enter_context(nc.psum_tensor("st_ps", [NPE, BL], f32))

        with nc.Block() as block, \
                nc.semaphore("const_sem") as const_sem, \
                nc.semaphore("dr0") as dr0, \
                nc.semaphore("dr1") as dr1, \
                nc.semaphore("dr2") as dr2, \
                nc.semaphore("dve_prod") as dve_prod, \
                nc.semaphore("gps_me") as gps_me, \
                nc.semaphore("pe_stat") as pe_stat, \
                nc.semaphore("act_done") as act_done, \
                nc.semaphore("act_fin") as act_fin, \
                nc.semaphore("dma_out") as dma_out:
            dring = (dr0, dr1, dr2)

            @block.sync
            def _(sync):
                sync.dma_start(out=oneh_sb[:], in_=oneh_d[:]).then_inc(const_sem, 16)
                sync.dma_start(out=gam_sb[:], in_=gam_d[:]).then_inc(const_sem, 16)
                for u in range(NUNIT):
                    if u >= 3:
                        sync.wait_ge(dve_prod, u - 2)
                        sync.wait_ge(gps_me, u - 2)
                        sync.wait_ge(pe_stat, u - 2)
                    sl = u % 3
                    rows = slice(u * P, (u + 1) * P)
                    for dst, src in ((r_in[sl], r_d), (v_in[sl], v_d),
                                     (l_in[sl], l_d), (e_in[sl], e_d),
                                     (m_in[sl], m_d)):
                        sync.dma_start(out=dst[:], in_=src[rows, :]) \
                            .then_inc(dring[sl], 16)
                sync.wait_ge(act_fin, 1)
                sync.dma_start(out=pes_d[:], in_=stats_sb[:]).then_inc(dma_out, 16)
                sync.wait_ge(act_done, 3 * NUNIT)
                sync.dma_start(out=cols_d[:], in_=cols[:]).then_inc(dma_out, 16)
                sync.wait_ge(dma_out, 32)

            @block.vector
            def _(dve):
                dve.wait_ge(const_sem, 32)
                for u in range(NUNIT):
                    sl = u % 3
                    pr = u % 2
                    dve.wait_ge(dring[sl], 80 * (u // 3 + 1))
                    if u >= 2:
                        # product ring WAR: PE stats / ACT squares of u-2 done
                        dve.wait_ge(pe_stat, u - 1)
                        dve.wait_ge(act_done, 3 * (u - 1))
                    init = 0.0 if u % NWIN == 0 else R_t[(u - 1) % 2][:, W - 1:W]
                    dve.tensor_tensor_scan(out=R_t[pr][:], data0=gam_sb[:],
                                           data1=r_in[sl][:], initial=init,
                                           op0=mult, op1=add)
                    dve.tensor_tensor(out=mR[pr][:], in0=m_in[sl][:], in1=R_t[pr][:], op=mult)
                    dve.tensor_tensor(out=mV[pr][:], in0=m_in[sl][:], in1=v_in[sl][:], op=mult)
                    dve.tensor_tensor(out=mL[pr][:], in0=m_in[sl][:], in1=l_in[sl][:], op=mult)
                    dve.tensor_tensor(out=mRV[pr][:], in0=mR[pr][:], in1=mV[pr][:], op=mult)
                    last = dve.tensor_tensor(out=mLR[pr][:], in0=mL[pr][:], in1=mR[pr][:], op=mult)
                    if not LV_ON_GPS[u]:
                        last = dve.tensor_tensor(out=mLV[pr][:], in0=mL[pr][:], in1=mV[pr][:], op=mult)
                    last.then_inc(dve_prod, 1)

            @block.gpsimd
            def _(gps):
                for u in range(NUNIT):
                    sl = u % 3
                    pr = u % 2
                    gps.wait_ge(dring[sl], 80 * (u // 3 + 1))
                    if u >= 2:
                        gps.wait_ge(act_done, 3 * (u - 1))   # mE WAR (SE of u-2)
                        gps.wait_ge(pe_stat, u - 1)          # mLV WAR
                    last = gps.tensor_tensor(out=mE[pr][:], in0=m_in[sl][:],
                                             in1=e_in[sl][:], op=mult)
                    if LV_ON_GPS[u]:
                        gps.wait_ge(dve_prod, u + 1)
                        last = gps.tensor_tensor(out=mLV[pr][:], in0=mL[pr][:],
                                                 in1=mV[pr][:], op=mult)
                    last.then_inc(gps_me, 1)

            @block.tensor
            def _(pe):
                pe.wait_ge(const_sem, 32)
                for u in range(NUNIT):
                    sl = u % 3
                    pr = u % 2
                    pe.wait_ge(dve_prod, u + 1)
                    if LV_ON_GPS[u]:
                        pe.wait_ge(gps_me, u + 1)
                    srcs = {"N": m_in[sl], "S1": mR[pr], "SV": mV[pr],
                            "SLP": mL[pr], "SRV": mRV[pr], "SLPR": mLR[pr],
                            "SLPV": mLV[pr]}
                    for k in range(NCH):
                        csl = slice(k * 512, (k + 1) * 512)
                        for j, stat in enumerate(PE_STATS):
                            mm = pe.matmul(
                                st_ps[:],
                                lhsT=oneh_sb[:, j * NPE:(j + 1) * NPE],
                                rhs=srcs[stat][:, csl],
                                start=(u == 0 and k == 0 and j == 0),
                                stop=(u == NUNIT - 1 and k == NCH - 1 and j == NPE - 1))
                    mm.then_inc(pe_stat, 1)

            @block.scalar
            def _(act):
                for u in range(NUNIT):
                    pr = u % 2
                    act.wait_ge(dve_prod, u + 1)
                    act.activation(sq[:], mR[pr][:], Square,
                                   accum_out=cols[:, u:u + 1]).then_inc(act_done, 1)
                    act.activation(sq[:], mV[pr][:], Square,
                                   accum_out=cols[:, NUNIT + u:NUNIT + u + 1]) \
                        .then_inc(act_done, 1)
                    act.wait_ge(gps_me, u + 1)
                    act.activation(sq[:], mE[pr][:], Copy,
                                   accum_out=cols[:, 2 * NUNIT + u:2 * NUNIT + u + 1]) \
                        .then_inc(act_done, 1)
                act.wait_ge(pe_stat, NUNIT)
                act.activation(stats_sb[:], st_ps[:], Copy).then_inc(act_fin, 1)

    return nc


def _get_program():
    if "nc" not in _cache:
        _cache["nc"] = _build_program()
    return _cache["nc"]


def _tile(x, dtype):
    """(T, BL) shard -> time-reversed, batch-major unit layout (NUNIT*P, W)."""
    # reverse time, transpose to (BL, T)
    xt = x[::-1, :].T                              # (BL, T)
    xt = xt.reshape(NBLK, P, NWIN, W)              # (j, p, w, t)
    xt = np.ascontiguousarray(xt.transpose(0, 2, 1, 3), dtype=np.float32)
    return xt.reshape(NUNIT * P, W).astype(dtype)


def _shard_inputs(inputs):
    import ml_dtypes

    bf16 = ml_dtypes.bfloat16
    fp8 = ml_dtypes.float8_e4m3
    r_t = fp8 if R_DT == "fp8" else bf16
    e_t = fp8 if E_DT == "fp8" else bf16

    r = np.asarray(inputs["rewards"], dtype=np.float32)
    v = np.asarray(inputs["value_estimates"], dtype=np.float32)
    lp = np.asarray(inputs["log_probs"], dtype=np.float32)
    e = np.asarray(inputs["entropies"], dtype=np.float32)
    m = inputs["to_include"].astype(np.float32)
    in_maps = []
    for c in range(NCORES):
        sl = slice(c * BL, (c + 1) * BL)
        in_maps.append({
            "rewards": _tile(r[:, sl], r_t),
            "value_estimates": _tile(v[:, sl], bf16),
            "log_probs": _tile(lp[:, sl], bf16),
            "entropies": _tile(e[:, sl], e_t),
            "to_include": _tile(m[:, sl], bf16),
        })
    return in_maps


def _execute(in_maps, trace=False):
    from concourse.bass_utils import run_bass_kernel_spmd

    nc = _get_program()
    return run_bass_kernel_spmd(nc, in_maps, list(range(NCORES)), trace=trace)


def _stats_from_results(results):
    tot = {name: 0.0 for name in PE_STATS + ("S2", "SV2", "SE")}
    for cm in results:
        pes = cm["pe_stats"].astype(np.float64)
        for j, name in enumerate(PE_STATS):
            tot[name] += pes[j].sum()
        ac = cm["acc_cols"].astype(np.float64)
        tot["S2"] += ac[:, 0:NUNIT].sum()
        tot["SV2"] += ac[:, NUNIT:2 * NUNIT].sum()
        tot["SE"] += ac[:, 2 * NUNIT:3 * NUNIT].sum()
    return tot


def _finalize(tot):
    N = tot["N"]; S1 = tot["S1"]; S2 = tot["S2"]
    SV = tot["SV"]; SRV = tot["SRV"]; SV2 = tot["SV2"]
    SLP = tot["SLP"]; SLPR = tot["SLPR"]; SLPV = tot["SLPV"]; SE = tot["SE"]
    mean = S1 / N
    q = S2 - 2.0 * mean * S1 + mean * mean * N   # sum(m*(R-mean)^2)
    var = q / (N - 1.0)
    s = np.sqrt(var) + EPS
    critic = q / (s * s) - 2.0 * (SRV - mean * SV) / s + SV2
    actor = -(SLPR - mean * SLP) / s + SLPV - ALPHA * SE
    return (np.float32(critic), np.float32(actor))


def kernel(**inputs):
    in_maps = _shard_inputs(inputs)
    res = _execute(in_maps, trace=False)
    tot = _stats_from_results(res.results)
    return _finalize(tot)


# revision 5
# speedup vs baseline: 1.5585x; 1.0956x over previous
"""Trainium2 Bass kernel for nn_ActorCritic loss_fn.

Strategy (v2, batch-major)
--------------------------
Both losses are polynomials in 10 masked global sums over the discounted
returns R, values V, log-probs L, entropies E and mask m:

    N    = sum(m)        S1   = sum(m*R)      S2  = sum(m*R^2)
    SV   = sum(m*V)      SRV  = sum(m*R*V)    SV2 = sum(m*V^2)
    SLP  = sum(m*L)      SLPR = sum(m*L*R)    SLPV= sum(m*L*V)
    SE   = sum(m*E)

Layout: batch on partitions, time along the free dim, TIME-REVERSED on the
host.  Each core gets 512 batch columns = 4 partition-blocks of 128; each
block's 8192 time steps split into 4 windows of 2048 -> 16 units of
(128, 2048) per core, streamed with a 3-deep ring.

Engines per unit:
  DVE : discounted-return scan as a native tensor_tensor_scan
        (state = gamma*state + r, fp32 state, gamma held as an f32 tile so
        the recurrence matches the reference bit-for-bit in structure),
        chained across windows via initial=prev[:, -1:]; then bf16 2x
        products mR, mV, mL, mRV, mLR (and mLV on some units).
  GPS : mE product every unit; mLV product on the other units.
  PE  : 7 stat reductions (N,S1,SV,SLP,SRV,SLPR,SLPV) as ones-column
        matmuls accumulating into one PSUM bank across all units.
  ACT : Square+accum_out for S2 and SV2, Copy+accum_out for SE
        (one column per unit; host sums).

Raw Bass with manual semaphores (walrus build allows one sync wait per
instruction -> standalone wait_ge).  Final scalar math on host in float64.
"""

import numpy as np
from contextlib import ExitStack

GAMMA = 0.99
ALPHA = 0.01
EPS = 1e-8

T = 8192
B = 4096
NCORES = 8
BL = B // NCORES          # 512 batch columns per core
P = 128                   # partition dim (batch block)
NBLK = BL // P            # 4 batch blocks
W = 2048                  # time window (free dim per unit)
NWIN = T // W             # 4 windows per block
NUNIT = NBLK * NWIN       # 16 units, u = j*NWIN + w
NCH = W // 512            # 4 matmul chunks per unit (moving max 512)

# dtypes for rewards / entropies ("bf16" or "fp8")
R_DT = "fp8"
E_DT = "fp8"
# GPS tensor_tensor ops grab the SBUF port pair that DVE's 2x perf mode
# needs (exclusive lock per instruction), so GPS gets ONLY the mE product,
# gated to run inside DVE's scan window (the scan is a 1x op on DVE's
# dedicated port).  Everything else stays on DVE at 2x.
LV_ON_GPS = [False for u in range(NUNIT)]

PE_STATS = ("N", "S1", "SV", "SLP", "SRV", "SLPR", "SLPV")
NPE = len(PE_STATS)

_cache = {}


def _build_program():
    import concourse.bass as bass
    import concourse.mybir as mybir
    import ml_dtypes

    dt = mybir.dt
    f32 = dt.float32
    bf16 = dt.bfloat16
    fp8 = dt.float8e4
    mult = mybir.AluOpType.mult
    add = mybir.AluOpType.add
    Square = mybir.ActivationFunctionType.Square
    Copy = mybir.ActivationFunctionType.Copy

    r_dt = fp8 if R_DT == "fp8" else bf16
    e_dt = fp8 if E_DT == "fp8" else bf16

    nc = bass.Bass()
    r_d = nc.dram_tensor("rewards", [NUNIT * P, W], r_dt, kind="ExternalInput")
    v_d = nc.dram_tensor("value_estimates", [NUNIT * P, W], bf16, kind="ExternalInput")
    l_d = nc.dram_tensor("log_probs", [NUNIT * P, W], bf16, kind="ExternalInput")
    e_d = nc.dram_tensor("entropies", [NUNIT * P, W], e_dt, kind="ExternalInput")
    m_d = nc.dram_tensor("to_include", [NUNIT * P, W], bf16, kind="ExternalInput")
    pes_d = nc.dram_tensor("pe_stats", [NPE, BL], f32, kind="ExternalOutput")
    cols_d = nc.dram_tensor("acc_cols", [P, 3 * NUNIT], f32, kind="ExternalOutput")

    # onehot matrix for stat matmuls: oneh[:, j*NPE + j] = 1
    oneh_np = np.zeros((P, NPE * NPE), dtype=np.float32)
    for j in range(NPE):
        oneh_np[:, j * NPE + j] = 1.0
    oneh_d = nc.inline_tensor(oneh_np.astype(ml_dtypes.bfloat16), "onehmat")
    # gamma tile for the scan (f32 so the recurrence coefficient is exact)
    gam_d = nc.inline_tensor(np.full((P, W), GAMMA, dtype=np.float32), "gammat")

    with ExitStack() as ctx:
        def sb(name, shape, dtype):
            return ctx.enter_context(nc.sbuf_tensor(name, list(shape), dtype))

        oneh_sb = sb("oneh_sb", (P, NPE * NPE), bf16)
        gam_sb = sb("gam_sb", (P, W), f32)
        r_in = [sb(f"r_in{i}", (P, W), r_dt) for i in range(3)]
        v_in = [sb(f"v_in{i}", (P, W), bf16) for i in range(3)]
        l_in = [sb(f"l_in{i}", (P, W), bf16) for i in range(3)]
        e_in = [sb(f"e_in{i}", (P, W), e_dt) for i in range(3)]
        m_in = [sb(f"m_in{i}", (P, W), bf16) for i in range(3)]
        R_t = [sb(f"R_t{i}", (P, W), bf16) for i in range(2)]
        mR = [sb(f"mR{i}", (P, W), bf16) for i in range(2)]
        mV = [sb(f"mV{i}", (P, W), bf16) for i in range(2)]
        mL = [sb(f"mL{i}", (P, W), bf16) for i in range(2)]
        mRV = [sb(f"mRV{i}", (P, W), bf16) for i in range(2)]
        mLR = [sb(f"mLR{i}", (P, W), bf16) for i in range(2)]
        mLV = [sb(f"mLV{i}", (P, W), bf16) for i in range(2)]
        mE = [sb(f"mE{i}", (P, W), bf16) for i in range(2)]
        sq = sb("sq", (P, W), bf16)
        cols = sb("cols", (P, 3 * NUNIT), f32)
        stats_sb = sb("stats_sb", (NPE, BL), f32)
        st_ps = ctx.enter_context(nc.psum_tensor("st_ps", [NPE, BL], f32))

        with nc.Block() as block, \
                nc.semaphore("const_sem") as const_sem, \
                nc.semaphore("dr0") as dr0, \
                nc.semaphore("dr1") as dr1, \
                nc.semaphore("dr2") as dr2, \
                nc.semaphore("dve_prod") as dve_prod, \
                nc.semaphore("gps_me") as gps_me, \
                nc.semaphore("pe_stat") as pe_stat, \
                nc.semaphore("act_done") as act_done, \
                nc.semaphore("act_se") as act_se, \
                nc.semaphore("act_fin") as act_fin, \
                nc.semaphore("dma_out") as dma_out:
            dring = (dr0, dr1, dr2)

            @block.sync
            def _(sync):
                sync.dma_start(out=oneh_sb[:], in_=oneh_d[:]).then_inc(const_sem, 16)
                sync.dma_start(out=gam_sb[:], in_=gam_d[:]).then_inc(const_sem, 16)
                for u in range(NUNIT):
                    if u >= 3:
                        sync.wait_ge(dve_prod, u - 2)
                        sync.wait_ge(gps_me, u - 2)
                        sync.wait_ge(pe_stat, u - 2)
                    sl = u % 3
                    rows = slice(u * P, (u + 1) * P)
                    for dst, src in ((r_in[sl], r_d), (v_in[sl], v_d),
                                     (l_in[sl], l_d), (e_in[sl], e_d),
                                     (m_in[sl], m_d)):
                        sync.dma_start(out=dst[:], in_=src[rows, :]) \
                            .then_inc(dring[sl], 16)
                sync.wait_ge(act_fin, 1)
                sync.dma_start(out=pes_d[:], in_=stats_sb[:]).then_inc(dma_out, 16)
                sync.wait_ge(act_done, 2 * NUNIT)
                sync.wait_ge(act_se, NUNIT)
                sync.dma_start(out=cols_d[:], in_=cols[:]).then_inc(dma_out, 16)
                sync.wait_ge(dma_out, 32)

            @block.vector
            def _(dve):
                dve.wait_ge(const_sem, 32)
                for u in range(NUNIT):
                    sl = u % 3
                    pr = u % 2
                    dve.wait_ge(dring[sl], 80 * (u // 3 + 1))
                    if u >= 2:
                        # product ring WAR: PE stats / ACT squares of u-2 done
                        dve.wait_ge(pe_stat, u - 1)
                        dve.wait_ge(act_done, 2 * (u - 1))
                    init = 0.0 if u % NWIN == 0 else R_t[(u - 1) % 2][:, W - 1:W]
                    dve.tensor_tensor_scan(out=R_t[pr][:], data0=gam_sb[:],
                                           data1=r_in[sl][:], initial=init,
                                           op0=mult, op1=add)
                    dve.tensor_tensor(out=mR[pr][:], in0=m_in[sl][:], in1=R_t[pr][:], op=mult)
                    dve.tensor_tensor(out=mV[pr][:], in0=m_in[sl][:], in1=v_in[sl][:], op=mult)
                    dve.tensor_tensor(out=mL[pr][:], in0=m_in[sl][:], in1=l_in[sl][:], op=mult)
                    dve.tensor_tensor(out=mRV[pr][:], in0=mR[pr][:], in1=mV[pr][:], op=mult)
                    last = dve.tensor_tensor(out=mLR[pr][:], in0=mL[pr][:], in1=mR[pr][:], op=mult)
                    if not LV_ON_GPS[u]:
                        last = dve.tensor_tensor(out=mLV[pr][:], in0=mL[pr][:], in1=mV[pr][:], op=mult)
                    last.then_inc(dve_prod, 1)

            @block.gpsimd
            def _(gps):
                for u in range(NUNIT):
                    sl = u % 3
                    pr = u % 2
                    gps.wait_ge(dring[sl], 80 * (u // 3 + 1))
                    if u >= 2:
                        gps.wait_ge(act_se, u - 1)           # mE WAR (SE of u-2)
                    # run inside DVE's scan(u+1) window: products(u) done
                    gps.wait_ge(dve_prod, u + 1)
                    gps.tensor_tensor(out=mE[pr][:], in0=m_in[sl][:],
                                      in1=e_in[sl][:], op=mult).then_inc(gps_me, 1)

            @block.tensor
            def _(pe):
                pe.wait_ge(const_sem, 32)
                for u in range(NUNIT):
                    sl = u % 3
                    pr = u % 2
                    pe.wait_ge(dve_prod, u + 1)
                    if LV_ON_GPS[u]:
                        pe.wait_ge(gps_me, u + 1)
                    srcs = {"N": m_in[sl], "S1": mR[pr], "SV": mV[pr],
                            "SLP": mL[pr], "SRV": mRV[pr], "SLPR": mLR[pr],
                            "SLPV": mLV[pr]}
                    for k in range(NCH):
                        csl = slice(k * 512, (k + 1) * 512)
                        for j, stat in enumerate(PE_STATS):
                            mm = pe.matmul(
                                st_ps[:],
                                lhsT=oneh_sb[:, j * NPE:(j + 1) * NPE],
                                rhs=srcs[stat][:, csl],
                                start=(u == 0 and k == 0 and j == 0),
                                stop=(u == NUNIT - 1 and k == NCH - 1 and j == NPE - 1))
                    mm.then_inc(pe_stat, 1)

            @block.scalar
            def _(act):
                for u in range(NUNIT):
                    pr = u % 2
                    act.wait_ge(dve_prod, u + 1)
                    act.activation(sq[:], mR[pr][:], Square,
                                   accum_out=cols[:, u:u + 1]).then_inc(act_done, 1)
                    act.activation(sq[:], mV[pr][:], Square,
                                   accum_out=cols[:, NUNIT + u:NUNIT + u + 1]) \
                        .then_inc(act_done, 1)
                    act.wait_ge(gps_me, u + 1)
                    act.activation(sq[:], mE[pr][:], Copy,
                                   accum_out=cols[:, 2 * NUNIT + u:2 * NUNIT + u + 1]) \
                        .then_inc(act_se, 1)
                act.wait_ge(pe_stat, NUNIT)
                act.activation(stats_sb[:], st_ps[:], Copy).then_inc(act_fin, 1)

    return nc


def _get_program():
    if "nc" not in _cache:
        _cache["nc"] = _build_program()
    return _cache["nc"]


def _tile(x, dtype):
    """(T, BL) shard -> time-reversed, batch-major unit layout (NUNIT*P, W)."""
    # reverse time, transpose to (BL, T)
    xt = x[::-1, :].T                              # (BL, T)
    xt = xt.reshape(NBLK, P, NWIN, W)              # (j, p, w, t)
    xt = np.ascontiguousarray(xt.transpose(0, 2, 1, 3), dtype=np.float32)
    return xt.reshape(NUNIT * P, W).astype(dtype)


def _shard_inputs(inputs):
    import ml_dtypes

    bf16 = ml_dtypes.bfloat16
    fp8 = ml_dtypes.float8_e4m3
    r_t = fp8 if R_DT == "fp8" else bf16
    e_t = fp8 if E_DT == "fp8" else bf16

    r = np.asarray(inputs["rewards"], dtype=np.float32)
    v = np.asarray(inputs["value_estimates"], dtype=np.float32)
    lp = np.asarray(inputs["log_probs"], dtype=np.float32)
    e = np.asarray(inputs["entropies"], dtype=np.float32)
    m = inputs["to_include"].astype(np.float32)
    in_maps = []
    for c in range(NCORES):
        sl = slice(c * BL, (c + 1) * BL)
        in_maps.append({
            "rewards": _tile(r[:, sl], r_t),
            "value_estimates": _tile(v[:, sl], bf16),
            "log_probs": _tile(lp[:, sl], bf16),
            "entropies": _tile(e[:, sl], e_t),
            "to_include": _tile(m[:, sl], bf16),
        })
    return in_maps


def _execute(in_maps, trace=False):
    from concourse.bass_utils import run_bass_kernel_spmd

    nc = _get_program()
    return run_bass_kernel_spmd(nc, in_maps, list(range(NCORES)), trace=trace)


def _stats_from_results(results):
    tot = {name: 0.0 for name in PE_STATS + ("S2", "SV2", "SE")}
    for cm in results:
        pes = cm["pe_stats"].astype(np.float64)
        for j, name in enumerate(PE_STATS):
            tot[name] += pes[j].sum()
        ac = cm["acc_cols"].astype(np.float64)
        tot["S2"] += ac[:, 0:NUNIT].sum()
        tot["SV2"] += ac[:, NUNIT:2 * NUNIT].sum()
        tot["SE"] += ac[:, 2 * NUNIT:3 * NUNIT].sum()
    return tot


def _finalize(tot):
    N = tot["N"]; S1 = tot["S1"]; S2 = tot["S2"]
    SV = tot["SV"]; SRV = tot["SRV"]; SV2 = tot["SV2"]
    SLP = tot["SLP"]; SLPR = tot["SLPR"]; SLPV = tot["SLPV"]; SE = tot["SE"]
    mean = S1 / N
    q = S2 - 2.0 * mean * S1 + mean * mean * N   # sum(m*(R-mean)^2)
    var = q / (N - 1.0)
    s = np.sqrt(var) + EPS
    critic = q / (s * s) - 2.0 * (SRV - mean * SV) / s + SV2
    actor = -(SLPR - mean * SLP) / s + SLPV - ALPHA * SE
    return (np.float32(critic), np.float32(actor))


def kernel(**inputs):
    in_maps = _shard_inputs(inputs)
    res = _execute(in_maps, trace=False)
    tot = _stats_from_results(res.results)
    return _finalize(tot)


# revision 6
# speedup vs baseline: 1.6919x; 1.0856x over previous
"""Trainium2 Bass kernel for nn_ActorCritic loss_fn.

Strategy (v2, batch-major)
--------------------------
Both losses are polynomials in 10 masked global sums over the discounted
returns R, values V, log-probs L, entropies E and mask m:

    N    = sum(m)        S1   = sum(m*R)      S2  = sum(m*R^2)
    SV   = sum(m*V)      SRV  = sum(m*R*V)    SV2 = sum(m*V^2)
    SLP  = sum(m*L)      SLPR = sum(m*L*R)    SLPV= sum(m*L*V)
    SE   = sum(m*E)

Layout: batch on partitions, time along the free dim, TIME-REVERSED on the
host.  Each core gets 512 batch columns = 4 partition-blocks of 128; each
block's 8192 time steps split into 4 windows of 2048 -> 16 units of
(128, 2048) per core, streamed with a 3-deep ring.

Engines per unit:
  DVE : discounted-return scan as a native tensor_tensor_scan
        (state = gamma*state + r, fp32 state, gamma held as an f32 tile so
        the recurrence matches the reference bit-for-bit in structure),
        chained across windows via initial=prev[:, -1:]; then bf16 2x
        products mR, mV, mL, mRV, mLR (and mLV on some units).
  GPS : mE product every unit; mLV product on the other units.
  PE  : 7 stat reductions (N,S1,SV,SLP,SRV,SLPR,SLPV) as ones-column
        matmuls accumulating into one PSUM bank across all units.
  ACT : Square+accum_out for S2 and SV2, Copy+accum_out for SE
        (one column per unit; host sums).

Raw Bass with manual semaphores (walrus build allows one sync wait per
instruction -> standalone wait_ge).  Final scalar math on host in float64.
"""

import numpy as np
from contextlib import ExitStack

GAMMA = 0.99
ALPHA = 0.01
EPS = 1e-8

T = 8192
B = 4096
NCORES = 8
BL = B // NCORES          # 512 batch columns per core
P = 128                   # partition dim (batch block)
NBLK = BL // P            # 4 batch blocks
W = 2048                  # time window (free dim per unit)
NWIN = T // W             # 4 windows per block
NUNIT = NBLK * NWIN       # 16 units, u = j*NWIN + w
NCH = W // 512            # 4 matmul chunks per unit (moving max 512)

# dtypes for rewards / entropies ("bf16" or "fp8")
R_DT = "fp8"
E_DT = "fp8"
# GPS tensor_tensor ops grab the SBUF port pair that DVE's 2x perf mode
# needs (exclusive lock per instruction), so GPS gets ONLY the mE product,
# gated to run inside DVE's scan window (the scan is a 1x op on DVE's
# dedicated port).  Everything else stays on DVE at 2x.
LV_ON_GPS = [False for u in range(NUNIT)]

PE_STATS = ("N", "S1", "SV", "SLP", "SRV", "SLPR", "SLPV")
NPE = len(PE_STATS)

_cache = {}


def _build_program():
    import concourse.bass as bass
    import concourse.mybir as mybir
    import ml_dtypes

    dt = mybir.dt
    f32 = dt.float32
    bf16 = dt.bfloat16
    fp8 = dt.float8e4
    mult = mybir.AluOpType.mult
    add = mybir.AluOpType.add
    Square = mybir.ActivationFunctionType.Square
    Copy = mybir.ActivationFunctionType.Copy

    r_dt = fp8 if R_DT == "fp8" else bf16
    e_dt = fp8 if E_DT == "fp8" else bf16

    nc = bass.Bass()
    r_d = nc.dram_tensor("rewards", [NUNIT * P, W], r_dt, kind="ExternalInput")
    v_d = nc.dram_tensor("value_estimates", [NUNIT * P, W], bf16, kind="ExternalInput")
    l_d = nc.dram_tensor("log_probs", [NUNIT * P, W], bf16, kind="ExternalInput")
    e_d = nc.dram_tensor("entropies", [NUNIT * P, W], e_dt, kind="ExternalInput")
    m_d = nc.dram_tensor("to_include", [NUNIT * P, W], bf16, kind="ExternalInput")
    pes_d = nc.dram_tensor("pe_stats", [NPE, BL], f32, kind="ExternalOutput")
    cols_d = nc.dram_tensor("acc_cols", [P, 3 * NUNIT], f32, kind="ExternalOutput")

    # onehot matrix for stat matmuls: oneh[:, j*NPE + j] = 1
    oneh_np = np.zeros((P, NPE * NPE), dtype=np.float32)
    for j in range(NPE):
        oneh_np[:, j * NPE + j] = 1.0
    oneh_d = nc.inline_tensor(oneh_np.astype(ml_dtypes.bfloat16), "onehmat")
    # gamma tile for the scan (f32 so the recurrence coefficient is exact)
    gam_d = nc.inline_tensor(np.full((P, W), GAMMA, dtype=np.float32), "gammat")

    with ExitStack() as ctx:
        def sb(name, shape, dtype):
            return ctx.enter_context(nc.sbuf_tensor(name, list(shape), dtype))

        oneh_sb = sb("oneh_sb", (P, NPE * NPE), bf16)
        gam_sb = sb("gam_sb", (P, W), f32)
        r_in = [sb(f"r_in{i}", (P, W), r_dt) for i in range(3)]
        v_in = [sb(f"v_in{i}", (P, W), bf16) for i in range(3)]
        l_in = [sb(f"l_in{i}", (P, W), bf16) for i in range(3)]
        e_in = [sb(f"e_in{i}", (P, W), e_dt) for i in range(3)]
        m_in = [sb(f"m_in{i}", (P, W), bf16) for i in range(3)]
        R_t = [sb(f"R_t{i}", (P, W), bf16) for i in range(2)]
        mR = [sb(f"mR{i}", (P, W), bf16) for i in range(2)]
        mV = [sb(f"mV{i}", (P, W), bf16) for i in range(2)]
        mL = [sb(f"mL{i}", (P, W), bf16) for i in range(2)]
        mRV = [sb(f"mRV{i}", (P, W), bf16) for i in range(2)]
        mLR = [sb(f"mLR{i}", (P, W), bf16) for i in range(2)]
        mLV = [sb(f"mLV{i}", (P, W), bf16) for i in range(2)]
        mE = [sb(f"mE{i}", (P, W), bf16) for i in range(2)]
        sq = sb("sq", (P, W), bf16)
        cols = sb("cols", (P, 3 * NUNIT), f32)
        stats_sb = sb("stats_sb", (NPE, BL), f32)
        st_ps = ctx.enter_context(nc.psum_tensor("st_ps", [NPE, BL], f32))

        with nc.Block() as block, \
                nc.semaphore("const_sem") as const_sem, \
                nc.semaphore("dr0") as dr0, \
                nc.semaphore("dr1") as dr1, \
                nc.semaphore("dr2") as dr2, \
                nc.semaphore("dve_prod") as dve_prod, \
                nc.semaphore("pe_stat") as pe_stat, \
                nc.semaphore("act_done") as act_done, \
                nc.semaphore("act_se") as act_se, \
                nc.semaphore("act_fin") as act_fin, \
                nc.semaphore("dma_out") as dma_out:
            dring = (dr0, dr1, dr2)

            @block.sync
            def _(sync):
                sync.dma_start(out=oneh_sb[:], in_=oneh_d[:]).then_inc(const_sem, 16)
                sync.dma_start(out=gam_sb[:], in_=gam_d[:]).then_inc(const_sem, 16)
                for u in range(NUNIT):
                    if u >= 3:
                        sync.wait_ge(dve_prod, u - 2)
                        sync.wait_ge(pe_stat, u - 2)
                    sl = u % 3
                    rows = slice(u * P, (u + 1) * P)
                    for dst, src in ((r_in[sl], r_d), (v_in[sl], v_d),
                                     (l_in[sl], l_d), (e_in[sl], e_d),
                                     (m_in[sl], m_d)):
                        sync.dma_start(out=dst[:], in_=src[rows, :]) \
                            .then_inc(dring[sl], 16)
                sync.wait_ge(act_fin, 1)
                sync.dma_start(out=pes_d[:], in_=stats_sb[:]).then_inc(dma_out, 16)
                sync.wait_ge(act_done, 2 * NUNIT)
                sync.wait_ge(act_se, NUNIT)
                sync.dma_start(out=cols_d[:], in_=cols[:]).then_inc(dma_out, 16)
                sync.wait_ge(dma_out, 32)

            @block.vector
            def _(dve):
                dve.wait_ge(const_sem, 32)
                for u in range(NUNIT):
                    sl = u % 3
                    pr = u % 2
                    dve.wait_ge(dring[sl], 80 * (u // 3 + 1))
                    if u >= 2:
                        # product ring WAR: PE stats / ACT squares / SE of u-2 done
                        dve.wait_ge(pe_stat, u - 1)
                        dve.wait_ge(act_done, 2 * (u - 1))
                        dve.wait_ge(act_se, u - 1)
                    init = 0.0 if u % NWIN == 0 else R_t[(u - 1) % 2][:, W - 1:W]
                    dve.tensor_tensor_scan(out=R_t[pr][:], data0=gam_sb[:],
                                           data1=r_in[sl][:], initial=init,
                                           op0=mult, op1=add)
                    dve.tensor_tensor(out=mR[pr][:], in0=m_in[sl][:], in1=R_t[pr][:], op=mult)
                    dve.tensor_tensor(out=mV[pr][:], in0=m_in[sl][:], in1=v_in[sl][:], op=mult)
                    dve.tensor_tensor(out=mL[pr][:], in0=m_in[sl][:], in1=l_in[sl][:], op=mult)
                    dve.tensor_tensor(out=mRV[pr][:], in0=mR[pr][:], in1=mV[pr][:], op=mult)
                    dve.tensor_tensor(out=mLR[pr][:], in0=mL[pr][:], in1=mR[pr][:], op=mult)
                    dve.tensor_tensor(out=mLV[pr][:], in0=mL[pr][:], in1=mV[pr][:], op=mult)
                    dve.tensor_tensor(out=mE[pr][:], in0=m_in[sl][:], in1=e_in[sl][:], op=mult) \
                        .then_inc(dve_prod, 1)

            @block.tensor
            def _(pe):
                pe.wait_ge(const_sem, 32)
                for u in range(NUNIT):
                    sl = u % 3
                    pr = u % 2
                    pe.wait_ge(dve_prod, u + 1)
                    srcs = {"N": m_in[sl], "S1": mR[pr], "SV": mV[pr],
                            "SLP": mL[pr], "SRV": mRV[pr], "SLPR": mLR[pr],
                            "SLPV": mLV[pr]}
                    for k in range(NCH):
                        csl = slice(k * 512, (k + 1) * 512)
                        for j, stat in enumerate(PE_STATS):
                            mm = pe.matmul(
                                st_ps[:],
                                lhsT=oneh_sb[:, j * NPE:(j + 1) * NPE],
                                rhs=srcs[stat][:, csl],
                                start=(u == 0 and k == 0 and j == 0),
                                stop=(u == NUNIT - 1 and k == NCH - 1 and j == NPE - 1))
                    mm.then_inc(pe_stat, 1)

            @block.scalar
            def _(act):
                for u in range(NUNIT):
                    pr = u % 2
                    act.wait_ge(dve_prod, u + 1)
                    act.activation(sq[:], mR[pr][:], Square,
                                   accum_out=cols[:, u:u + 1]).then_inc(act_done, 1)
                    act.activation(sq[:], mV[pr][:], Square,
                                   accum_out=cols[:, NUNIT + u:NUNIT + u + 1]) \
                        .then_inc(act_done, 1)
                    act.activation(sq[:], mE[pr][:], Copy,
                                   accum_out=cols[:, 2 * NUNIT + u:2 * NUNIT + u + 1]) \
                        .then_inc(act_se, 1)
                act.wait_ge(pe_stat, NUNIT)
                act.activation(stats_sb[:], st_ps[:], Copy).then_inc(act_fin, 1)

    return nc


def _get_program():
    if "nc" not in _cache:
        _cache["nc"] = _build_program()
    return _cache["nc"]


def _tile(x, dtype):
    """(T, BL) shard -> time-reversed, batch-major unit layout (NUNIT*P, W)."""
    # reverse time, transpose to (BL, T)
    xt = x[::-1, :].T                              # (BL, T)
    xt = xt.reshape(NBLK, P, NWIN, W)              # (j, p, w, t)
    xt = np.ascontiguousarray(xt.transpose(0, 2, 1, 3), dtype=np.float32)
    return xt.reshape(NUNIT * P, W).astype(dtype)


def _shard_inputs(inputs):
    import ml_dtypes

    bf16 = ml_dtypes.bfloat16
    fp8 = ml_dtypes.float8_e4m3
    r_t = fp8 if R_DT == "fp8" else bf16
    e_t = fp8 if E_DT == "fp8" else bf16

    r = np.asarray(inputs["rewards"], dtype=np.float32)
    v = np.asarray(inputs["value_estimates"], dtype=np.float32)
    lp = np.asarray(inputs["log_probs"], dtype=np.float32)
    e = np.asarray(inputs["entropies"], dtype=np.float32)
    m = inputs["to_include"].astype(np.float32)
    in_maps = []
    for c in range(NCORES):
        sl = slice(c * BL, (c + 1) * BL)
        in_maps.append({
            "rewards": _tile(r[:, sl], r_t),
            "value_estimates": _tile(v[:, sl], bf16),
            "log_probs": _tile(lp[:, sl], bf16),
            "entropies": _tile(e[:, sl], e_t),
            "to_include": _tile(m[:, sl], bf16),
        })
    return in_maps


def _execute(in_maps, trace=False):
    from concourse.bass_utils import run_bass_kernel_spmd

    nc = _get_program()
    return run_bass_kernel_spmd(nc, in_maps, list(range(NCORES)), trace=trace)


def _stats_from_results(results):
    tot = {name: 0.0 for name in PE_STATS + ("S2", "SV2", "SE")}
    for cm in results:
        pes = cm["pe_stats"].astype(np.float64)
        for j, name in enumerate(PE_STATS):
            tot[name] += pes[j].sum()
        ac = cm["acc_cols"].astype(np.float64)
        tot["S2"] += ac[:, 0:NUNIT].sum()
        tot["SV2"] += ac[:, NUNIT:2 * NUNIT].sum()
        tot["SE"] += ac[:, 2 * NUNIT:3 * NUNIT].sum()
    return tot


def _finalize(tot):
    N = tot["N"]; S1 = tot["S1"]; S2 = tot["S2"]
    SV = tot["SV"]; SRV = tot["SRV"]; SV2 = tot["SV2"]
    SLP = tot["SLP"]; SLPR = tot["SLPR"]; SLPV = tot["SLPV"]; SE = tot["SE"]
    mean = S1 / N
    q = S2 - 2.0 * mean * S1 + mean * mean * N   # sum(m*(R-mean)^2)
    var = q / (N - 1.0)
    s = np.sqrt(var) + EPS
    critic = q / (s * s) - 2.0 * (SRV - mean * SV) / s + SV2
    actor = -(SLPR - mean * SLP) / s + SLPV - ALPHA * SE
    return (np.float32(critic), np.float32(actor))


def kernel(**inputs):
    in_maps = _shard_inputs(inputs)
    res = _execute(in_maps, trace=False)
    tot = _stats_from_results(res.results)
    return _finalize(tot)


# revision 7
# speedup vs baseline: 1.8098x; 1.0697x over previous
"""Trainium2 Bass kernel for nn_ActorCritic loss_fn.

Strategy (v2, batch-major)
--------------------------
Both losses are polynomials in 10 masked global sums over the discounted
returns R, values V, log-probs L, entropies E and mask m:

    N    = sum(m)        S1   = sum(m*R)      S2  = sum(m*R^2)
    SV   = sum(m*V)      SRV  = sum(m*R*V)    SV2 = sum(m*V^2)
    SLP  = sum(m*L)      SLPR = sum(m*L*R)    SLPV= sum(m*L*V)
    SE   = sum(m*E)

Layout: batch on partitions, time along the free dim, TIME-REVERSED on the
host.  Each core gets 512 batch columns = 4 partition-blocks of 128; each
block's 8192 time steps split into 4 windows of 2048 -> 16 units of
(128, 2048) per core, streamed with a 3-deep ring.

Engines per unit:
  DVE : discounted-return scan as a native tensor_tensor_scan
        (state = gamma*state + r, fp32 state, gamma held as an f32 tile so
        the recurrence matches the reference bit-for-bit in structure),
        chained across windows via initial=prev[:, -1:]; then bf16 2x
        products mR, mV, mL, mRV, mLR (and mLV on some units).
  GPS : mE product every unit; mLV product on the other units.
  PE  : 7 stat reductions (N,S1,SV,SLP,SRV,SLPR,SLPV) as ones-column
        matmuls accumulating into one PSUM bank across all units.
  ACT : Square+accum_out for S2 and SV2, Copy+accum_out for SE
        (one column per unit; host sums).

Raw Bass with manual semaphores (walrus build allows one sync wait per
instruction -> standalone wait_ge).  Final scalar math on host in float64.
"""

import numpy as np
from contextlib import ExitStack

GAMMA = 0.99
ALPHA = 0.01
EPS = 1e-8

T = 8192
B = 4096
NCORES = 8
BL = B // NCORES          # 512 batch columns per core
P = 128                   # partition dim (batch block)
NBLK = BL // P            # 4 batch blocks
W = 2048                  # time window (free dim per unit)
NWIN = T // W             # 4 windows per block
NUNIT = NBLK * NWIN       # 16 units, u = j*NWIN + w
NCH = W // 512            # 4 matmul chunks per unit (moving max 512)

# dtypes for rewards / entropies ("bf16" or "fp8")
R_DT = "fp8"
E_DT = "bf16"
# GPS tensor_tensor ops grab the SBUF port pair that DVE's 2x perf mode
# needs (exclusive lock per instruction), so GPS gets ONLY the mE product,
# gated to run inside DVE's scan window (the scan is a 1x op on DVE's
# dedicated port).  Everything else stays on DVE at 2x.
LV_ON_GPS = [False for u in range(NUNIT)]

PE_STATS = ("N", "S1", "SV", "SLP", "SRV", "SLPR", "SLPV")
NPE = len(PE_STATS)

_cache = {}


def _build_program():
    import concourse.bass as bass
    import concourse.mybir as mybir
    import ml_dtypes

    dt = mybir.dt
    f32 = dt.float32
    bf16 = dt.bfloat16
    fp8 = dt.float8e4
    mult = mybir.AluOpType.mult
    add = mybir.AluOpType.add
    Square = mybir.ActivationFunctionType.Square
    Copy = mybir.ActivationFunctionType.Copy

    r_dt = fp8 if R_DT == "fp8" else bf16
    e_dt = fp8 if E_DT == "fp8" else bf16

    nc = bass.Bass()
    r_d = nc.dram_tensor("rewards", [NUNIT * P, W], r_dt, kind="ExternalInput")
    v_d = nc.dram_tensor("value_estimates", [NUNIT * P, W], bf16, kind="ExternalInput")
    l_d = nc.dram_tensor("log_probs", [NUNIT * P, W], bf16, kind="ExternalInput")
    e_d = nc.dram_tensor("entropies", [NUNIT * P, W], e_dt, kind="ExternalInput")
    m_d = nc.dram_tensor("to_include", [NUNIT * P, W], bf16, kind="ExternalInput")
    pes_d = nc.dram_tensor("pe_stats", [NPE, BL], f32, kind="ExternalOutput")
    cols_d = nc.dram_tensor("acc_cols", [P, 3 * NUNIT], f32, kind="ExternalOutput")

    # onehot matrix for stat matmuls: oneh[:, j*NPE + j] = 1
    oneh_np = np.zeros((P, NPE * NPE), dtype=np.float32)
    for j in range(NPE):
        oneh_np[:, j * NPE + j] = 1.0
    oneh_d = nc.inline_tensor(oneh_np.astype(ml_dtypes.bfloat16), "onehmat")
    # gamma tile for the scan (f32 so the recurrence coefficient is exact)
    gam_d = nc.inline_tensor(np.full((P, W), GAMMA, dtype=np.float32), "gammat")

    with ExitStack() as ctx:
        def sb(name, shape, dtype):
            return ctx.enter_context(nc.sbuf_tensor(name, list(shape), dtype))

        oneh_sb = sb("oneh_sb", (P, NPE * NPE), bf16)
        gam_sb = sb("gam_sb", (P, W), f32)
        r_in = [sb(f"r_in{i}", (P, W), r_dt) for i in range(3)]
        v_in = [sb(f"v_in{i}", (P, W), bf16) for i in range(3)]
        l_in = [sb(f"l_in{i}", (P, W), bf16) for i in range(3)]
        e_in = [sb(f"e_in{i}", (P, W), e_dt) for i in range(3)]
        m_in = [sb(f"m_in{i}", (P, W), bf16) for i in range(3)]
        R_t = [sb(f"R_t{i}", (P, W), bf16) for i in range(2)]
        mR = [sb(f"mR{i}", (P, W), bf16) for i in range(2)]
        mV = [sb(f"mV{i}", (P, W), bf16) for i in range(2)]
        mL = [sb(f"mL{i}", (P, W), bf16) for i in range(2)]
        mRV = [sb(f"mRV{i}", (P, W), bf16) for i in range(2)]
        mLR = [sb(f"mLR{i}", (P, W), bf16) for i in range(2)]
        mLV = [sb(f"mLV{i}", (P, W), bf16) for i in range(2)]
        mE = [sb(f"mE{i}", (P, W), bf16) for i in range(2)]
        sq = sb("sq", (P, W), bf16)
        cols = sb("cols", (P, 3 * NUNIT), f32)
        stats_sb = sb("stats_sb", (NPE, BL), f32)
        st_ps = ctx.enter_context(nc.psum_tensor("st_ps", [NPE, BL], f32))

        with nc.Block() as block, \
                nc.semaphore("const_sem") as const_sem, \
                nc.semaphore("dr0") as dr0, \
                nc.semaphore("dr1") as dr1, \
                nc.semaphore("dr2") as dr2, \
                nc.semaphore("dve_prod") as dve_prod, \
                nc.semaphore("pe_stat") as pe_stat, \
                nc.semaphore("act_done") as act_done, \
                nc.semaphore("act_se") as act_se, \
                nc.semaphore("act_fin") as act_fin, \
                nc.semaphore("dma_out") as dma_out:
            dring = (dr0, dr1, dr2)

            @block.sync
            def _(sync):
                sync.dma_start(out=oneh_sb[:], in_=oneh_d[:]).then_inc(const_sem, 16)
                sync.dma_start(out=gam_sb[:], in_=gam_d[:]).then_inc(const_sem, 16)
                for u in range(NUNIT):
                    if u >= 3:
                        sync.wait_ge(dve_prod, u - 2)
                        sync.wait_ge(pe_stat, u - 2)
                    sl = u % 3
                    rows = slice(u * P, (u + 1) * P)
                    for dst, src in ((r_in[sl], r_d), (v_in[sl], v_d),
                                     (l_in[sl], l_d), (e_in[sl], e_d),
                                     (m_in[sl], m_d)):
                        sync.dma_start(out=dst[:], in_=src[rows, :]) \
                            .then_inc(dring[sl], 16)
                sync.wait_ge(act_fin, 1)
                sync.dma_start(out=pes_d[:], in_=stats_sb[:]).then_inc(dma_out, 16)
                sync.wait_ge(act_done, 2 * NUNIT)
                sync.wait_ge(act_se, NUNIT)
                sync.dma_start(out=cols_d[:], in_=cols[:]).then_inc(dma_out, 16)
                sync.wait_ge(dma_out, 32)

            @block.vector
            def _(dve):
                dve.wait_ge(const_sem, 32)
                for u in range(NUNIT):
                    sl = u % 3
                    pr = u % 2
                    dve.wait_ge(dring[sl], 80 * (u // 3 + 1))
                    if u >= 2:
                        # product ring WAR: PE stats / ACT squares / SE of u-2 done
                        dve.wait_ge(pe_stat, u - 1)
                        dve.wait_ge(act_done, 2 * (u - 1))
                        dve.wait_ge(act_se, u - 1)
                    init = 0.0 if u % NWIN == 0 else R_t[(u - 1) % 2][:, W - 1:W]
                    dve.tensor_tensor_scan(out=R_t[pr][:], data0=gam_sb[:],
                                           data1=r_in[sl][:], initial=init,
                                           op0=mult, op1=add)
                    dve.tensor_tensor(out=mR[pr][:], in0=m_in[sl][:], in1=R_t[pr][:], op=mult)
                    dve.tensor_tensor(out=mV[pr][:], in0=m_in[sl][:], in1=v_in[sl][:], op=mult)
                    dve.tensor_tensor(out=mL[pr][:], in0=m_in[sl][:], in1=l_in[sl][:], op=mult)
                    dve.tensor_tensor(out=mRV[pr][:], in0=mR[pr][:], in1=mV[pr][:], op=mult)
                    dve.tensor_tensor(out=mLR[pr][:], in0=mL[pr][:], in1=mR[pr][:], op=mult)
                    dve.tensor_tensor(out=mLV[pr][:], in0=mL[pr][:], in1=mV[pr][:], op=mult)
                    dve.tensor_tensor(out=mE[pr][:], in0=m_in[sl][:], in1=e_in[sl][:], op=mult) \
                        .then_inc(dve_prod, 1)

            @block.tensor
            def _(pe):
                pe.wait_ge(const_sem, 32)
                for u in range(NUNIT):
                    sl = u % 3
                    pr = u % 2
                    pe.wait_ge(dve_prod, u + 1)
                    srcs = {"N": m_in[sl], "S1": mR[pr], "SV": mV[pr],
                            "SLP": mL[pr], "SRV": mRV[pr], "SLPR": mLR[pr],
                            "SLPV": mLV[pr]}
                    for k in range(NCH):
                        csl = slice(k * 512, (k + 1) * 512)
                        for j, stat in enumerate(PE_STATS):
                            mm = pe.matmul(
                                st_ps[:],
                                lhsT=oneh_sb[:, j * NPE:(j + 1) * NPE],
                                rhs=srcs[stat][:, csl],
                                start=(u == 0 and k == 0 and j == 0),
                                stop=(u == NUNIT - 1 and k == NCH - 1 and j == NPE - 1))
                    mm.then_inc(pe_stat, 1)

            @block.scalar
            def _(act):
                for u in range(NUNIT):
                    pr = u % 2
                    act.wait_ge(dve_prod, u + 1)
                    act.activation(sq[:], mR[pr][:], Square,
                                   accum_out=cols[:, u:u + 1]).then_inc(act_done, 1)
                    act.activation(sq[:], mV[pr][:], Square,
                                   accum_out=cols[:, NUNIT + u:NUNIT + u + 1]) \
                        .then_inc(act_done, 1)
                    act.activation(sq[:], mE[pr][:], Copy,
                                   accum_out=cols[:, 2 * NUNIT + u:2 * NUNIT + u + 1]) \
                        .then_inc(act_se, 1)
                act.wait_ge(pe_stat, NUNIT)
                act.activation(stats_sb[:], st_ps[:], Copy).then_inc(act_fin, 1)

    return nc


def _get_program():
    if "nc" not in _cache:
        _cache["nc"] = _build_program()
    return _cache["nc"]


def _tile(x, dtype):
    """(T, BL) shard -> time-reversed, batch-major unit layout (NUNIT*P, W)."""
    # reverse time, transpose to (BL, T)
    xt = x[::-1, :].T                              # (BL, T)
    xt = xt.reshape(NBLK, P, NWIN, W)              # (j, p, w, t)
    xt = np.ascontiguousarray(xt.transpose(0, 2, 1, 3), dtype=np.float32)
    return xt.reshape(NUNIT * P, W).astype(dtype)


def _shard_inputs(inputs):
    import ml_dtypes

    bf16 = ml_dtypes.bfloat16
    fp8 = ml_dtypes.float8_e4m3
    r_t = fp8 if R_DT == "fp8" else bf16
    e_t = fp8 if E_DT == "fp8" else bf16

    r = np.asarray(inputs["rewards"], dtype=np.float32)
    v = np.asarray(inputs["value_estimates"], dtype=np.float32)
    lp = np.asarray(inputs["log_probs"], dtype=np.float32)
    e = np.asarray(inputs["entropies"], dtype=np.float32)
    m = inputs["to_include"].astype(np.float32)
    in_maps = []
    for c in range(NCORES):
        sl = slice(c * BL, (c + 1) * BL)
        in_maps.append({
            "rewards": _tile(r[:, sl], r_t),
            "value_estimates": _tile(v[:, sl], bf16),
            "log_probs": _tile(lp[:, sl], bf16),
            "entropies": _tile(e[:, sl], e_t),
            "to_include": _tile(m[:, sl], bf16),
        })
    return in_maps


def _execute(in_maps, trace=False):
    from concourse.bass_utils import run_bass_kernel_spmd

    nc = _get_program()
    return run_bass_kernel_spmd(nc, in_maps, list(range(NCORES)), trace=trace)


def _stats_from_results(results):
    tot = {name: 0.0 for name in PE_STATS + ("S2", "SV2", "SE")}
    for cm in results:
        pes = cm["pe_stats"].astype(np.float64)
        for j, name in enumerate(PE_STATS):
            tot[name] += pes[j].sum()
        ac = cm["acc_cols"].astype(np.float64)
        tot["S2"] += ac[:, 0:NUNIT].sum()
        tot["SV2"] += ac[:, NUNIT:2 * NUNIT].sum()
        tot["SE"] += ac[:, 2 * NUNIT:3 * NUNIT].sum()
    return tot


def _finalize(tot):
    N = tot["N"]; S1 = tot["S1"]; S2 = tot["S2"]
    SV = tot["SV"]; SRV = tot["SRV"]; SV2 = tot["SV2"]
    SLP = tot["SLP"]; SLPR = tot["SLPR"]; SLPV = tot["SLPV"]; SE = tot["SE"]
    mean = S1 / N
    q = S2 - 2.0 * mean * S1 + mean * mean * N   # sum(m*(R-mean)^2)
    var = q / (N - 1.0)
    s = np.sqrt(var) + EPS
    critic = q / (s * s) - 2.0 * (SRV - mean * SV) / s + SV2
    actor = -(SLPR - mean * SLP) / s + SLPV - ALPHA * SE
    return (np.float32(critic), np.float32(actor))


def kernel(**inputs):
    in_maps = _shard_inputs(inputs)
    res = _execute(in_maps, trace=False)
    tot = _stats_from_results(res.results)
    return _finalize(tot)


# revision 9
# speedup vs baseline: 1.8639x; 1.0299x over previous
"""Trainium2 Bass kernel for nn_ActorCritic loss_fn.

Strategy (v2, batch-major)
--------------------------
Both losses are polynomials in 10 masked global sums over the discounted
returns R, values V, log-probs L, entropies E and mask m:

    N    = sum(m)        S1   = sum(m*R)      S2  = sum(m*R^2)
    SV   = sum(m*V)      SRV  = sum(m*R*V)    SV2 = sum(m*V^2)
    SLP  = sum(m*L)      SLPR = sum(m*L*R)    SLPV= sum(m*L*V)
    SE   = sum(m*E)

Layout: batch on partitions, time along the free dim, TIME-REVERSED on the
host.  Each core gets 512 batch columns = 4 partition-blocks of 128; each
block's 8192 time steps split into 4 windows of 2048 -> 16 units of
(128, 2048) per core, streamed with a 3-deep ring.

Engines per unit:
  DVE : discounted-return scan as a native tensor_tensor_scan
        (state = gamma*state + r, fp32 state, gamma held as an f32 tile so
        the recurrence matches the reference bit-for-bit in structure),
        chained across windows via initial=prev[:, -1:]; then bf16 2x
        products mR, mV, mL, mRV, mLR (and mLV on some units).
  GPS : mE product every unit; mLV product on the other units.
  PE  : 7 stat reductions (N,S1,SV,SLP,SRV,SLPR,SLPV) as ones-column
        matmuls accumulating into one PSUM bank across all units.
  ACT : Square+accum_out for S2 and SV2, Copy+accum_out for SE
        (one column per unit; host sums).

Raw Bass with manual semaphores (walrus build allows one sync wait per
instruction -> standalone wait_ge).  Final scalar math on host in float64.
"""

import numpy as np
from contextlib import ExitStack

GAMMA = 0.99
ALPHA = 0.01
EPS = 1e-8

T = 8192
B = 4096
NCORES = 8
BL = B // NCORES          # 512 batch columns per core
P = 128                   # partition dim (batch block)
NBLK = BL // P            # 4 batch blocks
W = 2048                  # time window (free dim per unit)
NWIN = T // W             # 4 windows per block
NUNIT = NBLK * NWIN       # 16 units, u = j*NWIN + w
NCH = W // 512            # 4 matmul chunks per unit (moving max 512)

# dtypes for rewards / entropies ("bf16" or "fp8")
R_DT = "fp8"
E_DT = "bf16"
# GPS tensor_tensor ops grab the SBUF port pair that DVE's 2x perf mode
# needs (exclusive lock per instruction), so GPS gets ONLY the mE product,
# gated to run inside DVE's scan window (the scan is a 1x op on DVE's
# dedicated port).  Everything else stays on DVE at 2x.
LV_ON_GPS = [False for u in range(NUNIT)]

PE_STATS = ("N", "S1", "SV", "SLP", "SRV", "SLPR", "SLPV")
NPE = len(PE_STATS)

_cache = {}


def _build_program():
    import concourse.bass as bass
    import concourse.mybir as mybir
    import ml_dtypes

    dt = mybir.dt
    f32 = dt.float32
    bf16 = dt.bfloat16
    fp8 = dt.float8e4
    mult = mybir.AluOpType.mult
    add = mybir.AluOpType.add
    Square = mybir.ActivationFunctionType.Square
    Copy = mybir.ActivationFunctionType.Copy

    r_dt = fp8 if R_DT == "fp8" else bf16
    e_dt = fp8 if E_DT == "fp8" else bf16

    nc = bass.Bass()
    r_d = nc.dram_tensor("rewards", [NUNIT * P, W], r_dt, kind="ExternalInput")
    v_d = nc.dram_tensor("value_estimates", [NUNIT * P, W], bf16, kind="ExternalInput")
    l_d = nc.dram_tensor("log_probs", [NUNIT * P, W], bf16, kind="ExternalInput")
    e_d = nc.dram_tensor("entropies", [NUNIT * P, W], e_dt, kind="ExternalInput")
    m_d = nc.dram_tensor("to_include", [NUNIT * P, W], bf16, kind="ExternalInput")
    pes_d = nc.dram_tensor("pe_stats", [NPE, BL], f32, kind="ExternalOutput")
    cols_d = nc.dram_tensor("acc_cols", [P, 3 * NUNIT], f32, kind="ExternalOutput")

    # onehot matrix for stat matmuls: oneh[:, j*NPE + j] = 1
    oneh_np = np.zeros((P, NPE * NPE), dtype=np.float32)
    for j in range(NPE):
        oneh_np[:, j * NPE + j] = 1.0
    oneh_d = nc.inline_tensor(oneh_np.astype(ml_dtypes.bfloat16), "onehmat")
    # gamma tile for the scan (f32 so the recurrence coefficient is exact)
    gam_d = nc.inline_tensor(np.full((P, W), GAMMA, dtype=np.float32), "gammat")

    with ExitStack() as ctx:
        def sb(name, shape, dtype):
            return ctx.enter_context(nc.sbuf_tensor(name, list(shape), dtype))

        oneh_sb = sb("oneh_sb", (P, NPE * NPE), bf16)
        gam_sb = sb("gam_sb", (P, W), f32)
        r_in = [sb(f"r_in{i}", (P, W), r_dt) for i in range(3)]
        v_in = [sb(f"v_in{i}", (P, W), bf16) for i in range(3)]
        l_in = [sb(f"l_in{i}", (P, W), bf16) for i in range(3)]
        e_in = [sb(f"e_in{i}", (P, W), e_dt) for i in range(3)]
        m_in = [sb(f"m_in{i}", (P, W), bf16) for i in range(3)]
        R_t = [sb(f"R_t{i}", (P, W), bf16) for i in range(2)]
        mR = [sb(f"mR{i}", (P, W), bf16) for i in range(2)]
        mV = [sb(f"mV{i}", (P, W), bf16) for i in range(2)]
        mL = [sb(f"mL{i}", (P, W), bf16) for i in range(2)]
        mRV = [sb(f"mRV{i}", (P, W), bf16) for i in range(2)]
        mLR = [sb(f"mLR{i}", (P, W), bf16) for i in range(2)]
        mLV = [sb(f"mLV{i}", (P, W), bf16) for i in range(2)]
        mE = [sb(f"mE{i}", (P, W), bf16) for i in range(2)]
        sq = sb("sq", (P, W), bf16)
        cols = sb("cols", (P, 3 * NUNIT), f32)
        stats_sb = sb("stats_sb", (NPE, BL), f32)
        st_ps = ctx.enter_context(nc.psum_tensor("st_ps", [NPE, BL], f32))

        with nc.Block() as block, \
                nc.semaphore("const_sem") as const_sem, \
                nc.semaphore("rsem0") as rsem0, \
                nc.semaphore("dr0") as dr0, \
                nc.semaphore("dr1") as dr1, \
                nc.semaphore("dr2") as dr2, \
                nc.semaphore("dve_p8") as dve_p8, \
                nc.semaphore("pe_stat") as pe_stat, \
                nc.semaphore("act_done") as act_done, \
                nc.semaphore("act_se") as act_se, \
                nc.semaphore("act_fin") as act_fin, \
                nc.semaphore("dma_out") as dma_out:
            dring = (dr0, dr1, dr2)
            # per-slot completion thresholds (unit 0's rewards use rsem0)
            thresh = {}
            cnt = [0, 0, 0]
            for u in range(NUNIT):
                cnt[u % 3] += 64 if u == 0 else 80
                thresh[u] = cnt[u % 3]

            @block.sync
            def _(sync):
                # unit 0 data first (startup latency), rewards separately
                sync.dma_start(out=r_in[0][:], in_=r_d[0:P, :]).then_inc(rsem0, 16)
                for dst, src in ((v_in[0], v_d), (l_in[0], l_d),
                                 (e_in[0], e_d), (m_in[0], m_d)):
                    sync.dma_start(out=dst[:], in_=src[0:P, :]).then_inc(dr0, 16)
                sync.dma_start(out=gam_sb[:], in_=gam_d[:]).then_inc(const_sem, 16)
                sync.dma_start(out=oneh_sb[:], in_=oneh_d[:]).then_inc(const_sem, 16)
                for u in range(1, NUNIT):
                    if u >= 3:
                        sync.wait_ge(dve_p8, 8 * (u - 2))
                        sync.wait_ge(pe_stat, u - 2)
                    sl = u % 3
                    rows = slice(u * P, (u + 1) * P)
                    for dst, src in ((r_in[sl], r_d), (v_in[sl], v_d),
                                     (l_in[sl], l_d), (e_in[sl], e_d),
                                     (m_in[sl], m_d)):
                        sync.dma_start(out=dst[:], in_=src[rows, :]) \
                            .then_inc(dring[sl], 16)
                sync.wait_ge(act_fin, 1)
                sync.dma_start(out=pes_d[:], in_=stats_sb[:]).then_inc(dma_out, 16)
                sync.wait_ge(act_done, 2 * NUNIT)
                sync.wait_ge(act_se, NUNIT)
                sync.dma_start(out=cols_d[:], in_=cols[:]).then_inc(dma_out, 16)
                sync.wait_ge(dma_out, 32)

            @block.vector
            def _(dve):
                dve.wait_ge(const_sem, 32)   # both const DMAs (order across queues not guaranteed)
                for u in range(NUNIT):
                    sl = u % 3
                    pr = u % 2
                    if u == 0:
                        dve.wait_ge(rsem0, 16)
                    else:
                        dve.wait_ge(dring[sl], thresh[u])
                    if u >= 2:
                        # product ring WAR: PE stats / ACT reads of u-2 done
                        dve.wait_ge(pe_stat, u - 1)
                        dve.wait_ge(act_done, 2 * (u - 1))
                        dve.wait_ge(act_se, u - 1)
                    init = 0.0 if u % NWIN == 0 else R_t[(u - 1) % 2][:, W - 1:W]
                    dve.tensor_tensor_scan(out=R_t[pr][:], data0=gam_sb[:],
                                           data1=r_in[sl][:], initial=init,
                                           op0=mult, op1=add).then_inc(dve_p8, 1)
                    if u == 0:
                        dve.wait_ge(dring[0], 64)
                    dve.tensor_tensor(out=mR[pr][:], in0=m_in[sl][:], in1=R_t[pr][:], op=mult).then_inc(dve_p8, 1)
                    dve.tensor_tensor(out=mV[pr][:], in0=m_in[sl][:], in1=v_in[sl][:], op=mult).then_inc(dve_p8, 1)
                    dve.tensor_tensor(out=mL[pr][:], in0=m_in[sl][:], in1=l_in[sl][:], op=mult).then_inc(dve_p8, 1)
                    dve.tensor_tensor(out=mRV[pr][:], in0=mR[pr][:], in1=mV[pr][:], op=mult).then_inc(dve_p8, 1)
                    dve.tensor_tensor(out=mLR[pr][:], in0=mL[pr][:], in1=mR[pr][:], op=mult).then_inc(dve_p8, 1)
                    dve.tensor_tensor(out=mLV[pr][:], in0=mL[pr][:], in1=mV[pr][:], op=mult).then_inc(dve_p8, 1)
                    dve.tensor_tensor(out=mE[pr][:], in0=m_in[sl][:], in1=e_in[sl][:], op=mult).then_inc(dve_p8, 1)

            @block.tensor
            def _(pe):
                pe.wait_ge(const_sem, 32)
                # stat j ready after dve_p8 >= 8u+1+prod_idx[j] (N needs only DMA)
                need = {"N": None, "S1": 2, "SV": 3, "SLP": 4,
                        "SRV": 5, "SLPR": 6, "SLPV": 7}
                for u in range(NUNIT):
                    sl = u % 3
                    pr = u % 2
                    srcs = {"N": m_in[sl], "S1": mR[pr], "SV": mV[pr],
                            "SLP": mL[pr], "SRV": mRV[pr], "SLPR": mLR[pr],
                            "SLPV": mLV[pr]}
                    for j, stat in enumerate(PE_STATS):
                        if need[stat] is None:
                            if u == 0:
                                pe.wait_ge(dring[0], 64)
                            else:
                                pe.wait_ge(dring[sl], thresh[u])
                        else:
                            pe.wait_ge(dve_p8, 8 * u + need[stat])
                        for k in range(NCH):
                            csl = slice(k * 512, (k + 1) * 512)
                            mm = pe.matmul(
                                st_ps[:],
                                lhsT=oneh_sb[:, j * NPE:(j + 1) * NPE],
                                rhs=srcs[stat][:, csl],
                                start=(u == 0 and j == 0 and k == 0),
                                stop=(u == NUNIT - 1 and j == NPE - 1 and k == NCH - 1))
                    mm.then_inc(pe_stat, 1)

            @block.scalar
            def _(act):
                for u in range(NUNIT):
                    pr = u % 2
                    act.wait_ge(dve_p8, 8 * u + 2)
                    act.activation(sq[:], mR[pr][:], Square,
                                   accum_out=cols[:, u:u + 1]).then_inc(act_done, 1)
                    act.wait_ge(dve_p8, 8 * u + 3)
                    act.activation(sq[:], mV[pr][:], Square,
                                   accum_out=cols[:, NUNIT + u:NUNIT + u + 1]) \
                        .then_inc(act_done, 1)
                    act.wait_ge(dve_p8, 8 * u + 8)
                    act.activation(sq[:], mE[pr][:], Copy,
                                   accum_out=cols[:, 2 * NUNIT + u:2 * NUNIT + u + 1]) \
                        .then_inc(act_se, 1)
                act.wait_ge(pe_stat, NUNIT)
                act.activation(stats_sb[:], st_ps[:], Copy).then_inc(act_fin, 1)

    return nc


def _get_program():
    if "nc" not in _cache:
        _cache["nc"] = _build_program()
    return _cache["nc"]


def _tile(x, dtype):
    """(T, BL) shard -> time-reversed, batch-major unit layout (NUNIT*P, W)."""
    # reverse time, transpose to (BL, T)
    xt = x[::-1, :].T                              # (BL, T)
    xt = xt.reshape(NBLK, P, NWIN, W)              # (j, p, w, t)
    xt = np.ascontiguousarray(xt.transpose(0, 2, 1, 3), dtype=np.float32)
    return xt.reshape(NUNIT * P, W).astype(dtype)


def _shard_inputs(inputs):
    import ml_dtypes

    bf16 = ml_dtypes.bfloat16
    fp8 = ml_dtypes.float8_e4m3
    r_t = fp8 if R_DT == "fp8" else bf16
    e_t = fp8 if E_DT == "fp8" else bf16

    r = np.asarray(inputs["rewards"], dtype=np.float32)
    v = np.asarray(inputs["value_estimates"], dtype=np.float32)
    lp = np.asarray(inputs["log_probs"], dtype=np.float32)
    e = np.asarray(inputs["entropies"], dtype=np.float32)
    m = inputs["to_include"].astype(np.float32)
    in_maps = []
    for c in range(NCORES):
        sl = slice(c * BL, (c + 1) * BL)
        in_maps.append({
            "rewards": _tile(r[:, sl], r_t),
            "value_estimates": _tile(v[:, sl], bf16),
            "log_probs": _tile(lp[:, sl], bf16),
            "entropies": _tile(e[:, sl], e_t),
            "to_include": _tile(m[:, sl], bf16),
        })
    return in_maps


def _execute(in_maps, trace=False):
    from concourse.bass_utils import run_bass_kernel_spmd

    nc = _get_program()
    return run_bass_kernel_spmd(nc, in_maps, list(range(NCORES)), trace=trace)


def _stats_from_results(results):
    tot = {name: 0.0 for name in PE_STATS + ("S2", "SV2", "SE")}
    for cm in results:
        pes = cm["pe_stats"].astype(np.float64)
        for j, name in enumerate(PE_STATS):
            tot[name] += pes[j].sum()
        ac = cm["acc_cols"].astype(np.float64)
        tot["S2"] += ac[:, 0:NUNIT].sum()
        tot["SV2"] += ac[:, NUNIT:2 * NUNIT].sum()
        tot["SE"] += ac[:, 2 * NUNIT:3 * NUNIT].sum()
    return tot


def _finalize(tot):
    N = tot["N"]; S1 = tot["S1"]; S2 = tot["S2"]
    SV = tot["SV"]; SRV = tot["SRV"]; SV2 = tot["SV2"]
    SLP = tot["SLP"]; SLPR = tot["SLPR"]; SLPV = tot["SLPV"]; SE = tot["SE"]
    mean = S1 / N
    q = S2 - 2.0 * mean * S1 + mean * mean * N   # sum(m*(R-mean)^2)
    var = q / (N - 1.0)
    s = np.sqrt(var) + EPS
    critic = q / (s * s) - 2.0 * (SRV - mean * SV) / s + SV2
    actor = -(SLPR - mean * SLP) / s + SLPV - ALPHA * SE
    return (np.float32(critic), np.float32(actor))


def kernel(**inputs):
    in_maps = _shard_inputs(inputs)
    res = _execute(in_maps, trace=False)
    tot = _stats_from_results(res.results)
    return _finalize(tot)


# revision 10
# speedup vs baseline: 1.8852x; 1.0114x over previous
"""Trainium2 Bass kernel for nn_ActorCritic loss_fn.

Strategy (v2, batch-major)
--------------------------
Both losses are polynomials in 10 masked global sums over the discounted
returns R, values V, log-probs L, entropies E and mask m:

    N    = sum(m)        S1   = sum(m*R)      S2  = sum(m*R^2)
    SV   = sum(m*V)      SRV  = sum(m*R*V)    SV2 = sum(m*V^2)
    SLP  = sum(m*L)      SLPR = sum(m*L*R)    SLPV= sum(m*L*V)
    SE   = sum(m*E)

Layout: batch on partitions, time along the free dim, TIME-REVERSED on the
host.  Each core gets 512 batch columns = 4 partition-blocks of 128; each
block's 8192 time steps split into 4 windows of 2048 -> 16 units of
(128, 2048) per core, streamed with a 3-deep ring.

Engines per unit:
  DVE : discounted-return scan as a native tensor_tensor_scan
        (state = gamma*state + r, fp32 state, gamma held as an f32 tile so
        the recurrence matches the reference bit-for-bit in structure),
        chained across windows via initial=prev[:, -1:]; then bf16 2x
        products mR, mV, mL, mRV, mLR (and mLV on some units).
  GPS : mE product every unit; mLV product on the other units.
  PE  : 7 stat reductions (N,S1,SV,SLP,SRV,SLPR,SLPV) as ones-column
        matmuls accumulating into one PSUM bank across all units.
  ACT : Square+accum_out for S2 and SV2, Copy+accum_out for SE
        (one column per unit; host sums).

Raw Bass with manual semaphores (walrus build allows one sync wait per
instruction -> standalone wait_ge).  Final scalar math on host in float64.
"""

import numpy as np
from contextlib import ExitStack

GAMMA = 0.99
ALPHA = 0.01
EPS = 1e-8

T = 8192
B = 4096
NCORES = 8
BL = B // NCORES          # 512 batch columns per core
P = 128                   # partition dim (batch block)
NBLK = BL // P            # 4 batch blocks
W = 2048                  # time window (free dim per unit)
NWIN = T // W             # 4 windows per block
NUNIT = NBLK * NWIN       # 16 units, u = j*NWIN + w
NCH = W // 512            # 4 matmul chunks per unit (moving max 512)

# dtypes for rewards / entropies ("bf16" or "fp8")
R_DT = "fp8"
E_DT = "bf16"
# GPS tensor_tensor ops grab the SBUF port pair that DVE's 2x perf mode
# needs (exclusive lock per instruction), so GPS gets ONLY the mE product,
# gated to run inside DVE's scan window (the scan is a 1x op on DVE's
# dedicated port).  Everything else stays on DVE at 2x.
LV_ON_GPS = [False for u in range(NUNIT)]

PE_STATS = ("N", "S1", "SV", "SLP", "SRV", "SLPR", "SLPV")
NPE = len(PE_STATS)

_cache = {}


def _build_program():
    import concourse.bass as bass
    import concourse.mybir as mybir
    import ml_dtypes

    dt = mybir.dt
    f32 = dt.float32
    bf16 = dt.bfloat16
    fp8 = dt.float8e4
    mult = mybir.AluOpType.mult
    add = mybir.AluOpType.add
    Square = mybir.ActivationFunctionType.Square
    Copy = mybir.ActivationFunctionType.Copy

    r_dt = fp8 if R_DT == "fp8" else bf16
    e_dt = fp8 if E_DT == "fp8" else bf16

    nc = bass.Bass()
    r_d = nc.dram_tensor("rewards", [NUNIT * P, W], r_dt, kind="ExternalInput")
    v_d = nc.dram_tensor("value_estimates", [NUNIT * P, W], bf16, kind="ExternalInput")
    l_d = nc.dram_tensor("log_probs", [NUNIT * P, W], bf16, kind="ExternalInput")
    e_d = nc.dram_tensor("entropies", [NUNIT * P, W], e_dt, kind="ExternalInput")
    m_d = nc.dram_tensor("to_include", [NUNIT * P, W], bf16, kind="ExternalInput")
    pes_d = nc.dram_tensor("pe_stats", [NPE, BL], f32, kind="ExternalOutput")
    cols_d = nc.dram_tensor("acc_cols", [P, 3 * NUNIT], f32, kind="ExternalOutput")

    # onehot matrix for stat matmuls: oneh[:, j*NPE + j] = 1
    oneh_np = np.zeros((P, NPE * NPE), dtype=np.float32)
    for j in range(NPE):
        oneh_np[:, j * NPE + j] = 1.0
    oneh_d = nc.inline_tensor(oneh_np.astype(ml_dtypes.bfloat16), "onehmat")
    # gamma tile for the scan (f32 so the recurrence coefficient is exact)
    gam_d = nc.inline_tensor(np.full((P, W), GAMMA, dtype=np.float32), "gammat")

    with ExitStack() as ctx:
        def sb(name, shape, dtype):
            return ctx.enter_context(nc.sbuf_tensor(name, list(shape), dtype))

        oneh_sb = sb("oneh_sb", (P, NPE * NPE), bf16)
        gam_sb = sb("gam_sb", (P, W), f32)
        r_in = [sb(f"r_in{i}", (P, W), r_dt) for i in range(3)]
        v_in = [sb(f"v_in{i}", (P, W), bf16) for i in range(3)]
        l_in = [sb(f"l_in{i}", (P, W), bf16) for i in range(3)]
        e_in = [sb(f"e_in{i}", (P, W), e_dt) for i in range(3)]
        m_in = [sb(f"m_in{i}", (P, W), bf16) for i in range(3)]
        R_t = [sb(f"R_t{i}", (P, W), bf16) for i in range(2)]
        mR = [sb(f"mR{i}", (P, W), bf16) for i in range(2)]
        mV = [sb(f"mV{i}", (P, W), bf16) for i in range(2)]
        mL = [sb(f"mL{i}", (P, W), bf16) for i in range(2)]
        mRV = [sb(f"mRV{i}", (P, W), bf16) for i in range(2)]
        mLR = [sb(f"mLR{i}", (P, W), bf16) for i in range(2)]
        mLV = [sb(f"mLV{i}", (P, W), bf16) for i in range(2)]
        mE = [sb(f"mE{i}", (P, W), bf16) for i in range(2)]
        sq = sb("sq", (P, W), bf16)
        cols = sb("cols", (P, 3 * NUNIT), f32)
        stats_sb = sb("stats_sb", (NPE, BL), f32)
        st_ps = ctx.enter_context(nc.psum_tensor("st_ps", [NPE, BL], f32))

        with nc.Block() as block, \
                nc.semaphore("const_sem") as const_sem, \
                nc.semaphore("rsem0") as rsem0, \
                nc.semaphore("dr0") as dr0, \
                nc.semaphore("dr1") as dr1, \
                nc.semaphore("dr2") as dr2, \
                nc.semaphore("dve_p8") as dve_p8, \
                nc.semaphore("pe_stat") as pe_stat, \
                nc.semaphore("act_done") as act_done, \
                nc.semaphore("act_se") as act_se, \
                nc.semaphore("act_fin") as act_fin, \
                nc.semaphore("dma_out") as dma_out:
            dring = (dr0, dr1, dr2)
            # per-slot completion thresholds (unit 0's rewards use rsem0)
            thresh = {}
            cnt = [0, 0, 0]
            for u in range(NUNIT):
                cnt[u % 3] += 64 if u == 0 else 80
                thresh[u] = cnt[u % 3]

            @block.sync
            def _(sync):
                # consts + unit-0 rewards first and ALONE, so the first scan
                # is not queued behind the bulk prefetch (queues share fairly)
                sync.dma_start(out=gam_sb[:], in_=gam_d[:]).then_inc(const_sem, 16)
                sync.dma_start(out=oneh_sb[:], in_=oneh_d[:]).then_inc(const_sem, 16)
                sync.dma_start(out=r_in[0][:], in_=r_d[0:P, :]).then_inc(rsem0, 16)
                sync.wait_ge(rsem0, 16)
                for dst, src in ((v_in[0], v_d), (l_in[0], l_d),
                                 (e_in[0], e_d), (m_in[0], m_d)):
                    sync.dma_start(out=dst[:], in_=src[0:P, :]).then_inc(dr0, 16)
                for u in range(1, NUNIT):
                    if u >= 3:
                        sync.wait_ge(dve_p8, 8 * (u - 2))
                        sync.wait_ge(pe_stat, u - 2)
                    sl = u % 3
                    rows = slice(u * P, (u + 1) * P)
                    for dst, src in ((r_in[sl], r_d), (v_in[sl], v_d),
                                     (l_in[sl], l_d), (e_in[sl], e_d),
                                     (m_in[sl], m_d)):
                        sync.dma_start(out=dst[:], in_=src[rows, :]) \
                            .then_inc(dring[sl], 16)
                sync.wait_ge(act_fin, 1)
                sync.dma_start(out=pes_d[:], in_=stats_sb[:]).then_inc(dma_out, 16)
                sync.wait_ge(act_done, 2 * NUNIT)
                sync.wait_ge(act_se, NUNIT)
                sync.dma_start(out=cols_d[:], in_=cols[:]).then_inc(dma_out, 16)
                sync.wait_ge(dma_out, 32)

            @block.vector
            def _(dve):
                dve.wait_ge(const_sem, 32)   # both const DMAs (order across queues not guaranteed)
                for u in range(NUNIT):
                    sl = u % 3
                    pr = u % 2
                    if u == 0:
                        dve.wait_ge(rsem0, 16)
                    else:
                        dve.wait_ge(dring[sl], thresh[u])
                    if u >= 2:
                        # product ring WAR: PE stats / ACT reads of u-2 done
                        dve.wait_ge(pe_stat, u - 1)
                        dve.wait_ge(act_done, 2 * (u - 1))
                        dve.wait_ge(act_se, u - 1)
                    init = 0.0 if u % NWIN == 0 else R_t[(u - 1) % 2][:, W - 1:W]
                    dve.tensor_tensor_scan(out=R_t[pr][:], data0=gam_sb[:],
                                           data1=r_in[sl][:], initial=init,
                                           op0=mult, op1=add).then_inc(dve_p8, 1)
                    if u == 0:
                        dve.wait_ge(dring[0], 64)
                    dve.tensor_tensor(out=mR[pr][:], in0=m_in[sl][:], in1=R_t[pr][:], op=mult).then_inc(dve_p8, 1)
                    dve.tensor_tensor(out=mV[pr][:], in0=m_in[sl][:], in1=v_in[sl][:], op=mult).then_inc(dve_p8, 1)
                    dve.tensor_tensor(out=mL[pr][:], in0=m_in[sl][:], in1=l_in[sl][:], op=mult).then_inc(dve_p8, 1)
                    dve.tensor_tensor(out=mRV[pr][:], in0=mR[pr][:], in1=mV[pr][:], op=mult).then_inc(dve_p8, 1)
                    dve.tensor_tensor(out=mLR[pr][:], in0=mL[pr][:], in1=mR[pr][:], op=mult).then_inc(dve_p8, 1)
                    dve.tensor_tensor(out=mLV[pr][:], in0=mL[pr][:], in1=mV[pr][:], op=mult).then_inc(dve_p8, 1)
                    dve.tensor_tensor(out=mE[pr][:], in0=m_in[sl][:], in1=e_in[sl][:], op=mult).then_inc(dve_p8, 1)

            @block.tensor
            def _(pe):
                pe.wait_ge(const_sem, 32)
                # stat j ready after dve_p8 >= 8u+1+prod_idx[j] (N needs only DMA)
                need = {"N": None, "S1": 2, "SV": 3, "SLP": 4,
                        "SRV": 5, "SLPR": 6, "SLPV": 7}
                for u in range(NUNIT):
                    sl = u % 3
                    pr = u % 2
                    srcs = {"N": m_in[sl], "S1": mR[pr], "SV": mV[pr],
                            "SLP": mL[pr], "SRV": mRV[pr], "SLPR": mLR[pr],
                            "SLPV": mLV[pr]}
                    for j, stat in enumerate(PE_STATS):
                        if need[stat] is None:
                            if u == 0:
                                pe.wait_ge(dring[0], 64)
                            else:
                                pe.wait_ge(dring[sl], thresh[u])
                        else:
                            pe.wait_ge(dve_p8, 8 * u + need[stat])
                        for k in range(NCH):
                            csl = slice(k * 512, (k + 1) * 512)
                            mm = pe.matmul(
                                st_ps[:],
                                lhsT=oneh_sb[:, j * NPE:(j + 1) * NPE],
                                rhs=srcs[stat][:, csl],
                                start=(u == 0 and j == 0 and k == 0),
                                stop=(u == NUNIT - 1 and j == NPE - 1 and k == NCH - 1))
                    mm.then_inc(pe_stat, 1)

            @block.scalar
            def _(act):
                for u in range(NUNIT):
                    pr = u % 2
                    act.wait_ge(dve_p8, 8 * u + 2)
                    act.activation(sq[:], mR[pr][:], Square,
                                   accum_out=cols[:, u:u + 1]).then_inc(act_done, 1)
                    act.wait_ge(dve_p8, 8 * u + 3)
                    act.activation(sq[:], mV[pr][:], Square,
                                   accum_out=cols[:, NUNIT + u:NUNIT + u + 1]) \
                        .then_inc(act_done, 1)
                    act.wait_ge(dve_p8, 8 * u + 8)
                    act.activation(sq[:], mE[pr][:], Copy,
                                   accum_out=cols[:, 2 * NUNIT + u:2 * NUNIT + u + 1]) \
                        .then_inc(act_se, 1)
                act.wait_ge(pe_stat, NUNIT)
                act.activation(stats_sb[:], st_ps[:], Copy).then_inc(act_fin, 1)

    return nc


def _get_program():
    if "nc" not in _cache:
        _cache["nc"] = _build_program()
    return _cache["nc"]


def _tile(x, dtype):
    """(T, BL) shard -> time-reversed, batch-major unit layout (NUNIT*P, W)."""
    # reverse time, transpose to (BL, T)
    xt = x[::-1, :].T                              # (BL, T)
    xt = xt.reshape(NBLK, P, NWIN, W)              # (j, p, w, t)
    xt = np.ascontiguousarray(xt.transpose(0, 2, 1, 3), dtype=np.float32)
    return xt.reshape(NUNIT * P, W).astype(dtype)


def _shard_inputs(inputs):
    import ml_dtypes

    bf16 = ml_dtypes.bfloat16
    fp8 = ml_dtypes.float8_e4m3
    r_t = fp8 if R_DT == "fp8" else bf16
    e_t = fp8 if E_DT == "fp8" else bf16

    r = np.asarray(inputs["rewards"], dtype=np.float32)
    v = np.asarray(inputs["value_estimates"], dtype=np.float32)
    lp = np.asarray(inputs["log_probs"], dtype=np.float32)
    e = np.asarray(inputs["entropies"], dtype=np.float32)
    m = inputs["to_include"].astype(np.float32)
    in_maps = []
    for c in range(NCORES):
        sl = slice(c * BL, (c + 1) * BL)
        in_maps.append({
            "rewards": _tile(r[:, sl], r_t),
            "value_estimates": _tile(v[:, sl], bf16),
            "log_probs": _tile(lp[:, sl], bf16),
            "entropies": _tile(e[:, sl], e_t),
            "to_include": _tile(m[:, sl], bf16),
        })
    return in_maps


def _execute(in_maps, trace=False):
    from concourse.bass_utils import run_bass_kernel_spmd

    nc = _get_program()
    return run_bass_kernel_spmd(nc, in_maps, list(range(NCORES)), trace=trace)


def _stats_from_results(results):
    tot = {name: 0.0 for name in PE_STATS + ("S2", "SV2", "SE")}
    for cm in results:
        pes = cm["pe_stats"].astype(np.float64)
        for j, name in enumerate(PE_STATS):
            tot[name] += pes[j].sum()
        ac = cm["acc_cols"].astype(np.float64)
        tot["S2"] += ac[:, 0:NUNIT].sum()
        tot["SV2"] += ac[:, NUNIT:2 * NUNIT].sum()
        tot["SE"] += ac[:, 2 * NUNIT:3 * NUNIT].sum()
    return tot


def _finalize(tot):
    N = tot["N"]; S1 = tot["S1"]; S2 = tot["S2"]
    SV = tot["SV"]; SRV = tot["SRV"]; SV2 = tot["SV2"]
    SLP = tot["SLP"]; SLPR = tot["SLPR"]; SLPV = tot["SLPV"]; SE = tot["SE"]
    mean = S1 / N
    q = S2 - 2.0 * mean * S1 + mean * mean * N   # sum(m*(R-mean)^2)
    var = q / (N - 1.0)
    s = np.sqrt(var) + EPS
    critic = q / (s * s) - 2.0 * (SRV - mean * SV) / s + SV2
    actor = -(SLPR - mean * SLP) / s + SLPV - ALPHA * SE
    return (np.float32(critic), np.float32(actor))


def kernel(**inputs):
    in_maps = _shard_inputs(inputs)
    res = _execute(in_maps, trace=False)
    tot = _stats_from_results(res.results)
    return _finalize(tot)
